# revision 24
# baseline (speedup 1.0000x reference)
"""GQA attention block (B=2,S=2048,D=4096,H=32,KV=8,HD=128) on 8 TRN2 NeuronCores.

Sharding: 8-way tensor parallel over heads. Core c owns kv-head c and q-heads
4c..4c+3 (wq/wk/wv column-sharded, wo row-sharded). The full-width Q/K
layernorms need cross-core mean/var, done with two tiny 32KB AllReduces.
Each core emits a partial [T,D] output; the host sums the 8 partials.

Schedule (v2 — restructured from the 1.15ms baseline):
  1. Projections for ALL 32 token tiles first; AR0 (tiles 0-15 stats) fires
     mid-phase and AR1 right after tile 31, so neither collective ever blocks
     the PE queue (the old layout wove AR-dependent transposes between the
     tail projection tiles and stalled the PE ~40us; AR1 also triggered ~90us
     late behind weave DVE work).
  2. Pre-attention block: LN postamble + rope + PE-transpose for all batch-0
     q/k tiles (DVE runs under the projection tail once AR0 lands).
     Rope operates on host-de-interleaved pairs (wq/wk rows permuted per head
     so (even,odd) components are contiguous) — 6 half-width DVE ops, no
     copies; q/k share the permutation so scores are unchanged. The LN apply
     runs on ACT (scale=rstd, bias=-mu*rstd per partition); the trivial
     affine (w=1,b=0, detected host-side) is skipped.
  3. Attention batch 0: head-PAIRED kt loop — each kT/v stationary load
     serves two heads' matmuls (the baseline paid ~4.3us/qblock of unhidden
     LDWEIGHTS); exp batched over both heads' score tiles ([128,2,512] PSUM,
     one ACT instr); softmax denominators via a bf16 add-tree on DVE ending
     in two accumulated [1,512] ones-matmuls into one PSUM bank (rows 0/32),
     replacing 8 ones-matmuls/qblock.
  4. wo projection for token tiles 0-15, with batch-1's rope as fills and its
     transposes interleaved into the last tiles (keeps HAM warm, tp shares
     PSUM with the 2-bank psO tiles).
  5. Attention batch 1 (same shape, no fills), then wo tiles 16-31.
All matmuls bf16 with f32 accumulation (fp8 DoubleRow measured 3.8e-2 rel
err per converted GEMM in simulation — over the 2e-2 budget, rejected).
"""

from contextlib import ExitStack

import numpy as np
import ml_dtypes

import concourse.bass as bass
import concourse.mybir as mybir
import concourse.tile as tile
from concourse import bacc
from concourse import bass_isa
from concourse import bass_utils
from concourse.bass import ts, ds
from concourse.masks import make_identity

BF16 = mybir.dt.bfloat16
F32 = mybir.dt.float32
AF = mybir.ActivationFunctionType
ALU = mybir.AluOpType
AX = mybir.AxisListType

B, S, D = 2, 2048, 4096
T = B * S                 # 4096 tokens
H, KV, HD = 32, 8, 128
NCORES = 8
HQ = H // NCORES          # 4 q heads per core
EQ = HQ * HD              # 512
NT = T // 128             # 32 token tiles
ND = D // 128             # 32 contraction chunks
ST = S // 128             # 16 seq tiles per batch
NQB = S // 512            # 4 q-blocks per (b,h)
EPS = 1e-5
SHIFT = 12.0              # constant softmax shift (scores verified < ~8)

PROFILE = False
LAST_EXEC_NS = None
LAST_TRACE_DIR = None
_CACHE = {}


def flat2(ap):  # flatten all free dims -> [P, prod(free)]
    n = len(ap.shape)
    if n == 2:
        return ap
    names = " ".join(f"d{i}" for i in range(n - 1))
    return ap.rearrange(f"p {names} -> p ({names})")


class _Ctx:
    pass


def _build(triv):
    key = ("nc", triv)
    if key in _CACHE:
        return _CACHE[key]
    nc = bacc.Bacc("TRN2", target_bir_lowering=False, debug=False,
                   num_devices=NCORES)

    g = _Ctx()
    g.triv = triv
    g.xT_d = nc.dram_tensor("xT", [128, ND, T], BF16, kind="ExternalInput")
    g.wqT_d = nc.dram_tensor("wqT", [128, ND, EQ], BF16, kind="ExternalInput")
    g.wkvT_d = nc.dram_tensor("wkvT", [128, ND, 2 * HD], BF16,
                              kind="ExternalInput")
    g.woT_d = nc.dram_tensor("woT", [128, HQ, D], BF16, kind="ExternalInput")
    g.cosq_d = nc.dram_tensor("cosq", [T, HQ, 64], BF16, kind="ExternalInput")
    g.sinq_d = nc.dram_tensor("sinq", [T, HQ, 64], BF16, kind="ExternalInput")
    g.cosk_d = nc.dram_tensor("cosk", [T, 64], BF16, kind="ExternalInput")
    g.sink_d = nc.dram_tensor("sink", [T, 64], BF16, kind="ExternalInput")
    if not triv:
        g.qw_d = nc.dram_tensor("qw", [1, EQ], F32, kind="ExternalInput")
        g.qb_d = nc.dram_tensor("qb", [1, EQ], F32, kind="ExternalInput")
        g.kw_d = nc.dram_tensor("kw", [1, HD], F32, kind="ExternalInput")
        g.kb_d = nc.dram_tensor("kb", [1, HD], F32, kind="ExternalInput")
    g.out_d = nc.dram_tensor("out", [T, D], BF16, kind="ExternalOutput")

    with tile.TileContext(nc) as tc:
        _emit(nc, tc, g)
    nc.compile()
    _CACHE[key] = nc
    return nc


def _emit(nc, tc, g):
    ctx = ExitStack()
    with ctx:
        cpool = ctx.enter_context(tc.tile_pool(name="cpool", bufs=1))
        persist = ctx.enter_context(tc.tile_pool(name="persist", bufs=1))
        ardram = ctx.enter_context(
            tc.tile_pool(name="ardram", bufs=1, space="DRAM"))

        # ---- constants ----
        g.ident = cpool.tile([128, 128], BF16, name="ident")
        make_identity(nc, g.ident[:])
        g.ones_r = cpool.tile([1, 128], F32, name="ones_r")   # K=1 bcast lhsT
        nc.vector.memset(g.ones_r[:], 1.0)
        g.ones_c = cpool.tile([128, 1], BF16, name="ones_c")  # psum-col lhsT
        nc.vector.memset(g.ones_c[:], 1.0)
        g.eps_c = cpool.tile([128, 1], F32, name="eps_c")
        nc.vector.memset(g.eps_c[:], EPS)
        g.shift_c = cpool.tile([128, 1], F32, name="shift_c")
        nc.vector.memset(g.shift_c[:], -SHIFT)

        if not g.triv:
            qw_sb = cpool.tile([1, EQ], F32, name="qw_sb")
            qb_sb = cpool.tile([1, EQ], F32, name="qb_sb")
            kw_sb = cpool.tile([1, HD], F32, name="kw_sb")
            kb_sb = cpool.tile([1, HD], F32, name="kb_sb")
            nc.sync.dma_start(qw_sb[:], g.qw_d.ap())
            nc.sync.dma_start(qb_sb[:], g.qb_d.ap())
            nc.sync.dma_start(kw_sb[:], g.kw_d.ap())
            nc.sync.dma_start(kb_sb[:], g.kb_d.ap())
            g.qwB = cpool.tile([128, HQ, 2, 64], F32, name="qwB")
            g.qbB = cpool.tile([128, HQ, 2, 64], F32, name="qbB")
            g.kwB = cpool.tile([128, 2, 64], F32, name="kwB")
            g.kbB = cpool.tile([128, 2, 64], F32, name="kbB")

        # persistent activations
        g.xq_raw = persist.tile([128, NT, HQ, 2, 64], BF16, name="xq_raw")
        g.xk_raw = persist.tile([128, NT, 2, 64], BF16, name="xk_raw")
        g.v_s = persist.tile([128, NT, HD], BF16, name="v_s")
        g.stats_s = persist.tile([128, NT, 4], F32, name="stats_s")
        g.stats_g = persist.tile([128, NT, 4], F32, name="stats_g")
        g.qT_s = persist.tile([128, HQ, T], BF16, name="qT_s")
        g.kT_s = persist.tile([128, T], BF16, name="kT_s")

        g.mu_q = cpool.tile([128, NT], F32, name="mu_q")
        g.rstd_q = cpool.tile([128, NT], F32, name="rstd_q")
        g.nmr_q = cpool.tile([128, NT], F32, name="nmr_q")
        g.mu_k = cpool.tile([128, NT], F32, name="mu_k")
        g.rstd_k = cpool.tile([128, NT], F32, name="rstd_k")
        g.nmr_k = cpool.tile([128, NT], F32, name="nmr_k")
        g.tmp_a = cpool.tile([128, NT], F32, name="tmp_a")
        g.tmp_b = cpool.tile([128, NT], F32, name="tmp_b")

        def all_reduce_half(hb):
            # dmas + trigger all on gpsimd: it blocks there harmlessly (its
            # next consumer, attention's partition_broadcast, runs after AR1)
            ar_in = ardram.tile([128, ST, 4], F32, tag=f"ar_in{hb}")
            ar_out = ardram.tile([128, ST, 4], F32, tag=f"ar_out{hb}",
                                 addr_space="Shared")
            nc.gpsimd.dma_start(ar_in[:], g.stats_s[:, ts(hb, ST)])
            nc.gpsimd.collective_compute(
                "AllReduce", ALU.add,
                replica_groups=[list(range(NCORES))],
                ins=[ar_in.opt()], outs=[ar_out.opt()])
            nc.gpsimd.dma_start(g.stats_g[:, ts(hb, ST)], ar_out[:])

        # -------- phase 1: q/k/v projection + stats + AllReduces ----------
        with tc.tile_pool(name="p1w", bufs=1) as p1w, \
             tc.tile_pool(name="p1x", bufs=3) as p1x, \
             tc.tile_pool(name="p1s", bufs=2) as p1s, \
             tc.tile_pool(name="p2w", bufs=1) as p2w, \
             tc.tile_pool(name="ps1", bufs=1, space="PSUM") as ps1:
            g.p2 = p2w

            if not g.triv:
                for bcsrc, bcdst, wid in ((qw_sb, g.qwB, EQ),
                                          (qb_sb, g.qbB, EQ),
                                          (kw_sb, g.kwB, HD),
                                          (kb_sb, g.kbB, HD)):
                    ps_bc = ps1.tile([128, wid], F32, tag="psbc", bufs=2)
                    nc.tensor.matmul(ps_bc[:], lhsT=g.ones_r[:],
                                     rhs=bcsrc[:], start=True, stop=True)
                    nc.scalar.copy(flat2(bcdst[:]), ps_bc[:])

            def load_xpair(tp):  # 256-token pairs: 512B runs, full DMA rate
                x_t = p1x.tile([128, ND, 256], BF16, tag="x_t", bufs=2)
                for j8 in range(0, ND, 8):
                    nc.sync.dma_start(x_t[:, ds(j8, 8), :],
                                      g.xT_d.ap()[:, ds(j8, 8), ts(tp, 256)])
                return x_t

            wq_s = p1w.tile([128, ND, EQ], BF16, name="wq_s")
            wkv_s = p1w.tile([128, ND, 2 * HD], BF16, name="wkv_s")
            # tile-0 operand DMAs interleaved chunk-wise with its matmul
            # emission so the first MMs only wait on the first chunk set
            x_pre0 = p1x.tile([128, ND, 256], BF16, tag="x_t", bufs=2,
                              name="x_pre0")

            def chunk_dmas(j8):
                nc.sync.dma_start(x_pre0[:, ds(j8, 8), :],
                                  g.xT_d.ap()[:, ds(j8, 8), ts(0, 256)])
                nc.sync.dma_start(wq_s[:, ds(j8, 8), :],
                                  g.wqT_d.ap()[:, ds(j8, 8), :])
                nc.sync.dma_start(wkv_s[:, ds(j8, 8), :],
                                  g.wkvT_d.ap()[:, ds(j8, 8), :])

            chunk_dmas(0)
            chunk_dmas(8)

            def qkv_tile(ti, x_pre=None, interleave=None):
                if ti % 2 == 0:
                    g.x_cur = x_pre if x_pre is not None else load_xpair(
                        ti // 2)
                x_t = g.x_cur
                tsl = ts(ti % 2, 128)
                psq = ps1.tile([128, EQ], F32, tag="psq", bufs=2)
                pskv = ps1.tile([128, 2 * HD], F32, tag="pskv", bufs=2)
                # k|v fused into one 256-wide moving operand so each x-chunk
                # stationary is loaded once and reused by both streams
                for j in range(ND):
                    nc.tensor.matmul(psq[:], lhsT=x_t[:, j, tsl],
                                     rhs=wq_s[:, j, :],
                                     start=(j == 0), stop=(j == ND - 1))
                    nc.tensor.matmul(pskv[:], lhsT=x_t[:, j, tsl],
                                     rhs=wkv_s[:, j, :],
                                     start=(j == 0), stop=(j == ND - 1))
                    if interleave and j in interleave:
                        interleave[j]()
                psk = pskv[:, 0:HD]
                psv = pskv[:, HD:2 * HD]
                nc.scalar.copy(flat2(g.xq_raw[:, ti]), psq[:])
                nc.scalar.copy(flat2(g.xk_raw[:, ti]), psk)
                nc.scalar.copy(g.v_s[:, ti, :], psv)
                scrap = p1s.tile([128, EQ], BF16, tag="scrap", bufs=2)
                nc.vector.tensor_reduce(out=g.stats_s[:, ti, 0:1],
                                        in_=psq[:], axis=AX.X, op=ALU.add)
                nc.scalar.activation(scrap[:], psq[:], AF.Square,
                                     accum_out=g.stats_s[:, ti, 1:2])
                scrapk = p1s.tile([128, HD], BF16, tag="scrapk", bufs=2)
                nc.vector.tensor_reduce(out=g.stats_s[:, ti, 2:3],
                                        in_=psk, axis=AX.X, op=ALU.add)
                nc.scalar.activation(scrapk[:], psk, AF.Square,
                                     accum_out=g.stats_s[:, ti, 3:4])

            qkv_tile(0, x_pre=x_pre0,
                     interleave={7: lambda: chunk_dmas(16),
                                 15: lambda: chunk_dmas(24)})
            for ti in range(1, ST):
                qkv_tile(ti)
            all_reduce_half(0)      # lands while tiles 16..31 project
            for ti in range(ST, 22):
                qkv_tile(ti)
            _postamble(nc, g, 0)    # DVE-only wait on AR0; PE unaffected
            # weave batch-0 rope (ACT/DVE only — no PE transposes, so a late
            # AR0 can never stall the projection matmul stream) into the
            # phase-1 tail where ACT/DVE are ~25% busy
            b0parts = [p for i in range(ST) for p in (("k", i), ("q", i))]
            for i, ti in enumerate(range(22, NT)):
                qkv_tile(ti)
                for wh, t2 in b0parts[i * 32 // 10:(i + 1) * 32 // 10]:
                    _ph2_rope(nc, g, t2, wh)
            all_reduce_half(1)      # lands during early attention b0

        # ------- phases 2..5: ph2(b0) | attn b0 | wo 0-15 + ph2(b1) |
        # -------               attn b1 | wo 16-31
        with tc.tile_pool(name="p34", bufs=1) as p34:
            g.oT_s = p34.tile([128, HQ, T], BF16, name="oT_s")
            g.woT_s = p34.tile([128, HQ, D], BF16, name="woT_s")
            nc.sync.dma_start(g.woT_s[:], g.woT_d.ap())

            # ---- pre-attention: batch-0 transposes (rope already done
            # ---- under the phase-1 tail); copies alternate DVE/ACT ----
            tp_order = ([("k", 0), ("k", 1)]
                        + [("q", i) for i in range(4)]
                        + [("k", i) for i in range(2, ST)]
                        + [("q", i) for i in range(4, ST)])
            with tc.tile_pool(name="tpa", bufs=1, space="PSUM") as tpp:
                g.tpp = tpp
                for n, (wh, t2) in enumerate(tp_order):
                    eng = nc.vector if n % 2 else nc.scalar
                    _ph2_tp(nc, g, t2, wh, ceng=eng)

            # ---- attention batch 0 (postamble(1) woven in) ----
            with tc.tile_pool(name="p3a", bufs=1) as p3, \
                 tc.tile_pool(name="ps3a", bufs=1, space="PSUM") as ps3:
                g.p3, g.ps3 = p3, ps3
                _attn_batch(nc, g, 0, {})
            # AR1 has certainly landed by now; first consumer is wo's fills
            _postamble(nc, g, 1)

            # ---- wo tiles 0-15, batch-1 ph2 rope as fills, transposes
            # ---- interleaved into the tail tiles ----
            b1_parts = ([("k", ti) for ti in range(ST, NT)]
                        + [("q", ti) for ti in range(ST, NT)])
            with tc.tile_pool(name="p2b", bufs=1) as p2b, \
                 tc.tile_pool(name="p4a", bufs=1) as p4, \
                 tc.tile_pool(name="ps4a", bufs=1, space="PSUM") as ps4:
                g.p2, g.p4, g.ps4, g.tpp = p2b, p4, ps4, ps4
                for ti in range(12):
                    ropes = b1_parts[ti * 8 // 3:(ti + 1) * 8 // 3]
                    _wo_tile(nc, g, ti)
                    for wh, t2 in ropes:
                        _ph2_rope(nc, g, t2, wh)
                for ti in range(12, ST):
                    tps = b1_parts[(ti - 12) * 8:(ti - 11) * 8]
                    _wo_tile(nc, g, ti, tp_parts=tps)

            # ---- attention batch 1 ----
            with tc.tile_pool(name="p3b", bufs=1) as p3, \
                 tc.tile_pool(name="ps3b", bufs=1, space="PSUM") as ps3:
                g.p3, g.ps3 = p3, ps3
                _attn_batch(nc, g, 1, {})

            # ---- wo tiles 16-31 ----
            with tc.tile_pool(name="p4b", bufs=1) as p4, \
                 tc.tile_pool(name="ps4b", bufs=1, space="PSUM") as ps4:
                g.p4, g.ps4 = p4, ps4
                for ti in range(ST, NT):
                    _wo_tile(nc, g, ti)


def _postamble(nc, g, hb):
    """mu/rstd/-mu*rstd for one AllReduce half (token tiles hb*ST..)."""
    sl = ts(hb, ST)

    def stat(k):
        return g.stats_g[:, sl, k:k + 1].rearrange("p t s -> p (t s)")

    for (mu_t, rstd_t, nmr_t, s0, s1, e_full) in (
            (g.mu_q, g.rstd_q, g.nmr_q, 0, 1, D),
            (g.mu_k, g.rstd_k, g.nmr_k, 2, 3, KV * HD)):
        nc.vector.tensor_scalar_mul(mu_t[:, sl], stat(s0), 1.0 / e_full)
        nc.vector.tensor_scalar_mul(g.tmp_a[:, sl], stat(s1), 1.0 / e_full)
        nc.vector.tensor_mul(g.tmp_b[:, sl], mu_t[:, sl], mu_t[:, sl])
        nc.vector.tensor_sub(g.tmp_a[:, sl], g.tmp_a[:, sl], g.tmp_b[:, sl])
        nc.scalar.activation(g.tmp_b[:, sl], g.tmp_a[:, sl], AF.Sqrt,
                             bias=g.eps_c[:])
        nc.vector.reciprocal(rstd_t[:, sl], g.tmp_b[:, sl])
        nc.vector.scalar_tensor_tensor(
            out=nmr_t[:, sl], in0=mu_t[:, sl], scalar=-1.0,
            in1=rstd_t[:, sl], op0=ALU.mult, op1=ALU.mult)


def _ph2_rope(nc, g, ti, which):
    """LN apply (on ACT) + de-interleaved rope (6 half-width DVE ops),
    written IN PLACE over the raw projection tile (dead after this)."""
    p2 = g.p2
    if which == "q":
        raw = g.xq_raw[:, ti]                 # [128, HQ, 2, 64]
        mu_t, rstd_t, nmr_t = g.mu_q, g.rstd_q, g.nmr_q
        wB = g.qwB if not g.triv else None
        bB = g.qbB if not g.triv else None
        nh = HQ
        cos_t = p2.tile([128, HQ, 1, 64], BF16, tag="cosq", bufs=2)
        sin_t = p2.tile([128, HQ, 1, 64], BF16, tag="sinq", bufs=2)
        # ACT-queue trigger: a WAR wait here must not block the sync
        # queue's x/weight prefetch stream
        nc.scalar.dma_start(cos_t[:].rearrange("p h o s -> p h (o s)"),
                            g.cosq_d.ap()[ts(ti, 128)])
        nc.scalar.dma_start(sin_t[:].rearrange("p h o s -> p h (o s)"),
                            g.sinq_d.ap()[ts(ti, 128)])
        xn_t = p2.tile([128, HQ, 2, 64], BF16, tag="xnq", bufs=2)
        mshape = [128, HQ, 1, 64]
        x0, x1 = xn_t[:, :, 0:1, :], xn_t[:, :, 1:2, :]
        rp0, rp1 = raw[:, :, 0:1, :], raw[:, :, 1:2, :]
    else:
        raw = g.xk_raw[:, ti]                 # [128, 2, 64]
        mu_t, rstd_t, nmr_t = g.mu_k, g.rstd_k, g.nmr_k
        wB = g.kwB if not g.triv else None
        bB = g.kbB if not g.triv else None
        nh = 1
        cos_t = p2.tile([128, 1, 64], BF16, tag="cosk", bufs=2)
        sin_t = p2.tile([128, 1, 64], BF16, tag="sink", bufs=2)
        nc.scalar.dma_start(flat2(cos_t[:]), g.cosk_d.ap()[ts(ti, 128)])
        nc.scalar.dma_start(flat2(sin_t[:]), g.sink_d.ap()[ts(ti, 128)])
        xn_t = p2.tile([128, 2, 64], BF16, tag="xnk", bufs=2)
        mshape = [128, 1, 64]
        x0, x1 = xn_t[:, 0:1, :], xn_t[:, 1:2, :]
        rp0, rp1 = raw[:, 0:1, :], raw[:, 1:2, :]

    # xn = (raw - mu) * rstd on ACT: scale=rstd, bias=-mu*rstd
    nc.scalar.activation(flat2(xn_t[:]), flat2(raw), AF.Identity,
                         bias=nmr_t[:, ti:ti + 1],
                         scale=rstd_t[:, ti:ti + 1])
    if wB is not None:
        nc.vector.tensor_mul(xn_t[:], xn_t[:], wB[:])
        nc.vector.tensor_add(xn_t[:], xn_t[:], bB[:])
    mA = p2.tile(mshape, BF16, tag=f"mA{which}", bufs=2)
    mB = p2.tile(mshape, BF16, tag=f"mB{which}", bufs=2)
    nc.vector.tensor_mul(mA[:], x0, cos_t[:])
    nc.vector.tensor_mul(mB[:], x1, sin_t[:])
    nc.vector.tensor_sub(rp0, mA[:], mB[:])
    mC = p2.tile(mshape, BF16, tag=f"mC{which}", bufs=2)
    mD = p2.tile(mshape, BF16, tag=f"mD{which}", bufs=2)
    nc.vector.tensor_mul(mC[:], x0, sin_t[:])
    nc.vector.tensor_mul(mD[:], x1, cos_t[:])
    nc.vector.tensor_add(rp1, mC[:], mD[:])


def _ph2_tp(nc, g, ti, which, ceng=None):
    """PE-transpose rope output [t,(hd)] -> [hd,t]; copies on ACT/DVE."""
    nh = HQ if which == "q" else 1
    for h in range(nh):
        src = g.xq_raw[:, ti, h] if which == "q" else g.xk_raw[:, ti]
        tp_ps = g.tpp.tile([128, 128], BF16, tag="tp", bufs=2)
        nc.tensor.transpose(tp_ps[:], flat2(src), g.ident[:])
        dst = (g.qT_s[:, h, ts(ti, 128)] if which == "q"
               else g.kT_s[:, ts(ti, 128)])
        if ceng is nc.vector:
            nc.vector.tensor_copy(dst, tp_ps[:])
        else:
            nc.scalar.copy(dst, tp_ps[:])


def _attn_batch(nc, g, b, fill):
    """Attention for one batch; head-paired kt loop. fill maps (qb, hp) ->
    "post2" emitted after that head-pair's epilogue."""
    p3, ps3 = g.p3, g.ps3
    for qb in range(NQB):
        for hp in range(2):
            h0, h1 = 2 * hp, 2 * hp + 1
            qsl = ds(b * S + qb * 512, 512)
            psVs = [ps3.tile([128, 512], F32, tag="psV", bufs=2,
                             name=f"psV{hi}") for hi in range(2)]

            def psb_mm(kt):
                t = ps3.tile([128, 2, 512], F32, tag="psB", bufs=3)
                for hi, h in enumerate((h0, h1)):
                    nc.tensor.matmul(
                        t[:, hi], lhsT=g.kT_s[:, ds(b * S + kt * 128, 128)],
                        rhs=g.qT_s[:, h, qsl], start=True, stop=True)
                return t

            psBs = [psb_mm(0)]
            L1s = ([], [])
            L2s = ([], [])
            L3s = ([], [])
            prevT = None
            for kt in range(ST):
                attnT = p3.tile([128, 2, 512], BF16, tag="attnT", bufs=4)
                nc.scalar.activation(flat2(attnT[:]), flat2(psBs[kt][:]),
                                     AF.Exp, bias=g.shift_c[:])
                if kt + 1 < ST:
                    psBs.append(psb_mm(kt + 1))
                for hi in range(2):
                    nc.tensor.matmul(psVs[hi][:],
                                     lhsT=g.v_s[:, b * ST + kt, :],
                                     rhs=attnT[:, hi], start=(kt == 0),
                                     stop=(kt == ST - 1))
                if kt % 2 == 1:
                    # bf16 add-tree for softmax denominators (kills 7 of the
                    # baseline's 8 ones-matmuls per qblock)
                    for hi in range(2):
                        L1 = p3.tile([128, 512], BF16, tag="L1", bufs=4)
                        nc.vector.tensor_add(L1[:], prevT[:, hi],
                                             attnT[:, hi])
                        L1s[hi].append(L1)
                        if len(L1s[hi]) % 2 == 0:
                            L2 = p3.tile([128, 512], BF16, tag="L2", bufs=3)
                            nc.vector.tensor_add(L2[:], L1s[hi][-2][:],
                                                 L1s[hi][-1][:])
                            L2s[hi].append(L2)
                            if len(L2s[hi]) % 2 == 0:
                                L3 = p3.tile([128, 512], BF16, tag="L3",
                                             bufs=4)
                                nc.vector.tensor_add(L3[:], L2s[hi][-2][:],
                                                     L2s[hi][-1][:])
                                L3s[hi].append(L3)
                prevT = attnT

            # epilogue: final tree add, then the partition reduce on gpsimd
            # (keeps PE out of the softmax sums entirely and frees the psum
            # bank the ones-matmuls needed, buying psB bufs=3)
            for hi, h in enumerate((h0, h1)):
                L4 = p3.tile([128, 512], BF16, tag="L4", bufs=2)
                nc.vector.tensor_add(L4[:], L3s[hi][0][:], L3s[hi][1][:])
                den = p3.tile([128, 512], F32, tag="den", bufs=2)
                nc.gpsimd.partition_all_reduce(den[:], L4[:], channels=128,
                                               reduce_op=bass_isa.ReduceOp.add)
                rc_sb = p3.tile([128, 512], F32, tag="rc_sb", bufs=2)
                nc.vector.reciprocal_approx_fast(out=rc_sb[:], in_=den[:])
                nc.vector.tensor_mul(g.oT_s[:, h, qsl], psVs[hi][:],
                                     rc_sb[:])
            if fill.get((qb, hp)) == "post2":
                _postamble(nc, g, 1)


def _wo_tile(nc, g, ti, tp_parts=()):
    """Output projection for one 128-token tile (four 1024-wide quads,
    2-bank psO so transposes can share PSUM). tp_parts: batch-1 ph2
    transposes interleaved between quads to keep HAM warm."""
    p4, ps4 = g.p4, g.ps4
    tp_parts = list(tp_parts)
    for quad in range(4):
        psO = ps4.tile([128, 2, 512], F32, tag="psO", bufs=3)
        for h in range(HQ):
            for nb in range(2):
                nc.tensor.matmul(
                    psO[:, nb], lhsT=g.oT_s[:, h, ts(ti, 128)],
                    rhs=g.woT_s[:, h, ds(quad * 1024 + nb * 512, 512)],
                    start=(h == 0), stop=(h == HQ - 1))
        outst = p4.tile([128, 2, 512], BF16, tag="outst", bufs=3)
        if quad % 2 == 0:
            nc.vector.tensor_copy(flat2(outst[:]), flat2(psO[:]))
        else:
            nc.scalar.copy(flat2(outst[:]), flat2(psO[:]))
        nc.sync.dma_start(g.out_d.ap()[ts(ti, 128), ds(quad * 1024, 1024)],
                          flat2(outst[:]))
        for wh, t2 in tp_parts[quad * 2:quad * 2 + 2]:
            _ph2_tp(nc, g, t2, wh)


def _host_inputs(x, freqs_cis, wq, wk, wv, wo, q_norm_w, q_norm_b,
                 k_norm_w, k_norm_b, triv):
    bf = ml_dtypes.bfloat16
    f32 = np.float32
    x = np.asarray(x, f32)
    freqs_cis = np.asarray(freqs_cis, f32)
    wq = np.asarray(wq, f32)
    wk = np.asarray(wk, f32)
    wv = np.asarray(wv, f32)
    wo = np.asarray(wo, f32)
    q_norm_w = np.asarray(q_norm_w, f32)
    q_norm_b = np.asarray(q_norm_b, f32)
    k_norm_w = np.asarray(k_norm_w, f32)
    k_norm_b = np.asarray(k_norm_b, f32)

    xf = np.ascontiguousarray(x.reshape(T, D))
    xT_r = np.ascontiguousarray(
        xf.T.reshape(ND, 128, T).transpose(1, 0, 2)).astype(bf)

    # rope de-interleave: within each head, rows (even dims | odd dims)
    perm = np.concatenate([np.arange(0, HD, 2), np.arange(1, HD, 2)])
    scale = 1.0 / np.sqrt(np.float32(HD))
    cos = freqs_cis[:, :, 0]          # [S, 64]
    sin = freqs_cis[:, :, 1]
    cos2 = np.concatenate([cos] * B, 0)   # [T, 64]
    sin2 = np.concatenate([sin] * B, 0)
    cosq = np.ascontiguousarray(np.broadcast_to(
        (cos2 * scale)[:, None], (T, HQ, 64))).astype(bf)
    sinq = np.ascontiguousarray(np.broadcast_to(
        (sin2 * scale)[:, None], (T, HQ, 64))).astype(bf)
    cosk = np.ascontiguousarray(cos2).astype(bf)
    sink = np.ascontiguousarray(sin2).astype(bf)

    in_maps = []
    for c in range(NCORES):
        wq_c = wq[c * EQ:(c + 1) * EQ].reshape(HQ, HD, D)[:, perm].reshape(
            EQ, D)
        wk_c = wk[c * HD:(c + 1) * HD][perm]          # [128, D]
        wv_c = wv[c * HD:(c + 1) * HD]
        wo_c = wo[:, c * EQ:(c + 1) * EQ]             # [D, 512]
        wqT_r = np.ascontiguousarray(
            wq_c.T.reshape(ND, 128, EQ).transpose(1, 0, 2)).astype(bf)
        wkT_r = np.ascontiguousarray(
            wk_c.T.reshape(ND, 128, HD).transpose(1, 0, 2)).astype(bf)
        wvT_r = np.ascontiguousarray(
            wv_c.T.reshape(ND, 128, HD).transpose(1, 0, 2)).astype(bf)
        wkvT_r = np.ascontiguousarray(
            np.concatenate([wkT_r, wvT_r], axis=2))
        woT_r = np.ascontiguousarray(
            wo_c.T.reshape(HQ, 128, D).transpose(1, 0, 2)).astype(bf)
        im = {
            "xT": xT_r, "wqT": wqT_r, "wkvT": wkvT_r,
            "woT": woT_r, "cosq": cosq, "sinq": sinq, "cosk": cosk,
            "sink": sink,
        }
        if not triv:
            qp = np.concatenate([p + c * EQ for p in
                                 [h * HD + perm for h in range(HQ)]])
            im["qw"] = q_norm_w[qp].astype(f32).reshape(1, EQ)
            im["qb"] = q_norm_b[qp].astype(f32).reshape(1, EQ)
            im["kw"] = k_norm_w[c * HD + perm].astype(f32).reshape(1, HD)
            im["kb"] = k_norm_b[c * HD + perm].astype(f32).reshape(1, HD)
        in_maps.append(im)
    return in_maps


def _run_profiled(nc, in_maps):
    """bass2jax execute wrapped in an NRT profile capture; returns
    (results, max exec_time_ns across cores, trace_dir)."""
    import ctypes
    import glob
    import tempfile

    import jax
    from concourse import bass2jax
    import gauge.profiler
    from concourse.bass_utils import FishPath

    lib = ctypes.CDLL("/opt/axon/libaxon_pjrt.so")
    if not hasattr(lib, "axon_start_nrt_profile"):
        results = bass2jax.run_bass_via_pjrt(nc, in_maps, n_cores=NCORES)
        return results, None, None
    lib.axon_start_nrt_profile.argtypes = [ctypes.POINTER(ctypes.c_int64),
                                           ctypes.c_size_t]
    lib.axon_start_nrt_profile.restype = ctypes.c_int64
    lib.axon_stop_nrt_profile.argtypes = [ctypes.c_char_p]
    lib.axon_stop_nrt_profile.restype = ctypes.c_int64

    jax.devices()
    # warm-up execution: loads the NEFF and aligns core dispatch so the
    # profiled run isn't polluted by first-run start skew
    bass2jax.run_bass_via_pjrt(nc, in_maps, n_cores=NCORES)
    neff_dir = tempfile.mkdtemp(prefix="bassprof_")
    rc = lib.axon_start_nrt_profile(None, 0)
    if rc != 0:
        raise RuntimeError(f"axon_start_nrt_profile rc={rc}")
    try:
        results = bass2jax.run_bass_via_pjrt(nc, in_maps, n_cores=NCORES)
    finally:
        n = lib.axon_stop_nrt_profile(neff_dir.encode())
        print(f"profile: {n} ntff file(s) in {neff_dir}")
    ntffs = glob.glob(neff_dir + "/*_body*.ntff")
    if not ntffs:
        return results, None, None
    profile = gauge.profiler.Profile(
        profile_path=FishPath(neff_dir), kernel_dev_mode=True,
        profile_on_exit=False, bass_kernel=nc.m,
        offline_processing=True, fname="*_body*")
    exec_ns = None
    try:
        prs = profile.to_perfetto(model_index=list(range(NCORES)))
        times = [pr.exec_time_ns for pr in prs if pr.exec_time_ns]
        exec_ns = max(times) if times else None
    except Exception as e:  # profile parse best-effort
        print("profile parse failed:", e)
    return results, exec_ns, neff_dir


def kernel(x, freqs_cis, wq, wk, wv, wo, q_norm_w, q_norm_b,
           k_norm_w, k_norm_b):
    global LAST_EXEC_NS, LAST_TRACE_DIR
    triv = (np.allclose(np.asarray(q_norm_w), 1.0)
            and np.allclose(np.asarray(q_norm_b), 0.0)
            and np.allclose(np.asarray(k_norm_w), 1.0)
            and np.allclose(np.asarray(k_norm_b), 0.0))
    nc = _build(triv)
    in_maps = _host_inputs(x, freqs_cis, wq, wk, wv, wo,
                           q_norm_w, q_norm_b, k_norm_w, k_norm_b, triv)
    if PROFILE:
        results, LAST_EXEC_NS, LAST_TRACE_DIR = _run_profiled(nc, in_maps)
    else:
        res = bass_utils.run_bass_kernel_spmd(
            nc, in_maps, core_ids=list(range(NCORES)))
        results = res.results
        LAST_EXEC_NS = res.exec_time_ns
    acc = np.zeros((T, D), np.float32)
    for r in results:
        acc += np.asarray(r["out"], np.float32)
    return acc.reshape(B, S, D)


# revision 26
# speedup vs baseline: 1.0223x; 1.0223x over previous
"""GQA attention block (B=2,S=2048,D=4096,H=32,KV=8,HD=128) on 8 TRN2 NeuronCores.

Sharding: 8-way tensor parallel over heads. Core c owns kv-head c and q-heads
4c..4c+3 (wq/wk/wv column-sharded, wo row-sharded). The full-width Q/K
layernorms need cross-core mean/var, done with two tiny 32KB AllReduces.
Each core emits a partial [T,D] output; the host sums the 8 partials.

Schedule (v2 — restructured from the 1.15ms baseline):
  1. Projections for ALL 32 token tiles first; AR0 (tiles 0-15 stats) fires
     mid-phase and AR1 right after tile 31, so neither collective ever blocks
     the PE queue (the old layout wove AR-dependent transposes between the
     tail projection tiles and stalled the PE ~40us; AR1 also triggered ~90us
     late behind weave DVE work).
  2. Pre-attention block: LN postamble + rope + PE-transpose for all batch-0
     q/k tiles (DVE runs under the projection tail once AR0 lands).
     Rope operates on host-de-interleaved pairs (wq/wk rows permuted per head
     so (even,odd) components are contiguous) — 6 half-width DVE ops, no
     copies; q/k share the permutation so scores are unchanged. The LN apply
     runs on ACT (scale=rstd, bias=-mu*rstd per partition); the trivial
     affine (w=1,b=0, detected host-side) is skipped.
  3. Attention batch 0: head-PAIRED kt loop — each kT/v stationary load
     serves two heads' matmuls (the baseline paid ~4.3us/qblock of unhidden
     LDWEIGHTS); exp batched over both heads' score tiles ([128,2,512] PSUM,
     one ACT instr); softmax denominators via a bf16 add-tree on DVE ending
     in two accumulated [1,512] ones-matmuls into one PSUM bank (rows 0/32),
     replacing 8 ones-matmuls/qblock.
  4. wo projection for token tiles 0-15, with batch-1's rope as fills and its
     transposes interleaved into the last tiles (keeps HAM warm, tp shares
     PSUM with the 2-bank psO tiles).
  5. Attention batch 1 (same shape, no fills), then wo tiles 16-31.
All matmuls bf16 with f32 accumulation (fp8 DoubleRow measured 3.8e-2 rel
err per converted GEMM in simulation — over the 2e-2 budget, rejected).
"""

from contextlib import ExitStack

import numpy as np
import ml_dtypes

import concourse.bass as bass
import concourse.mybir as mybir
import concourse.tile as tile
from concourse import bacc
from concourse import bass_isa
from concourse import bass_utils
from concourse.bass import ts, ds
from concourse.masks import make_identity

BF16 = mybir.dt.bfloat16
F32 = mybir.dt.float32
AF = mybir.ActivationFunctionType
ALU = mybir.AluOpType
AX = mybir.AxisListType

B, S, D = 2, 2048, 4096
T = B * S                 # 4096 tokens
H, KV, HD = 32, 8, 128
NCORES = 8
HQ = H // NCORES          # 4 q heads per core
EQ = HQ * HD              # 512
NT = T // 128             # 32 token tiles
ND = D // 128             # 32 contraction chunks
ST = S // 128             # 16 seq tiles per batch
NQB = S // 512            # 4 q-blocks per (b,h)
EPS = 1e-5
SHIFT = 12.0              # constant softmax shift (scores verified < ~8)

PROFILE = False
LAST_EXEC_NS = None
LAST_TRACE_DIR = None
_CACHE = {}


def flat2(ap):  # flatten all free dims -> [P, prod(free)]
    n = len(ap.shape)
    if n == 2:
        return ap
    names = " ".join(f"d{i}" for i in range(n - 1))
    return ap.rearrange(f"p {names} -> p ({names})")


class _Ctx:
    pass


def _build(triv):
    key = ("nc", triv)
    if key in _CACHE:
        return _CACHE[key]
    nc = bacc.Bacc("TRN2", target_bir_lowering=False, debug=False,
                   num_devices=NCORES)

    g = _Ctx()
    g.triv = triv
    g.xT_d = nc.dram_tensor("xT", [128, ND, T], BF16, kind="ExternalInput")
    g.wqT_d = nc.dram_tensor("wqT", [128, ND, EQ], BF16, kind="ExternalInput")
    g.wkvT_d = nc.dram_tensor("wkvT", [128, ND, 2 * HD], BF16,
                              kind="ExternalInput")
    g.woT_d = nc.dram_tensor("woT", [128, HQ, D], BF16, kind="ExternalInput")
    g.cosq_d = nc.dram_tensor("cosq", [T, HQ, 64], BF16, kind="ExternalInput")
    g.sinq_d = nc.dram_tensor("sinq", [T, HQ, 64], BF16, kind="ExternalInput")
    g.cosk_d = nc.dram_tensor("cosk", [T, 64], BF16, kind="ExternalInput")
    g.sink_d = nc.dram_tensor("sink", [T, 64], BF16, kind="ExternalInput")
    if not triv:
        g.qw_d = nc.dram_tensor("qw", [1, EQ], F32, kind="ExternalInput")
        g.qb_d = nc.dram_tensor("qb", [1, EQ], F32, kind="ExternalInput")
        g.kw_d = nc.dram_tensor("kw", [1, HD], F32, kind="ExternalInput")
        g.kb_d = nc.dram_tensor("kb", [1, HD], F32, kind="ExternalInput")
    g.out_d = nc.dram_tensor("out", [T, D], BF16, kind="ExternalOutput")

    with tile.TileContext(nc) as tc:
        _emit(nc, tc, g)
    nc.compile()
    _CACHE[key] = nc
    return nc


def _emit(nc, tc, g):
    ctx = ExitStack()
    with ctx:
        cpool = ctx.enter_context(tc.tile_pool(name="cpool", bufs=1))
        persist = ctx.enter_context(tc.tile_pool(name="persist", bufs=1))
        ardram = ctx.enter_context(
            tc.tile_pool(name="ardram", bufs=1, space="DRAM"))

        # ---- constants ----
        g.ident = cpool.tile([128, 128], BF16, name="ident")
        make_identity(nc, g.ident[:])
        g.ones_r = cpool.tile([1, 128], F32, name="ones_r")   # K=1 bcast lhsT
        nc.vector.memset(g.ones_r[:], 1.0)
        g.ones_c = cpool.tile([128, 1], BF16, name="ones_c")  # psum-col lhsT
        nc.vector.memset(g.ones_c[:], 1.0)
        g.eps_c = cpool.tile([128, 1], F32, name="eps_c")
        nc.vector.memset(g.eps_c[:], EPS)
        g.shift_c = cpool.tile([128, 1], F32, name="shift_c")
        nc.vector.memset(g.shift_c[:], -SHIFT)

        if not g.triv:
            qw_sb = cpool.tile([1, EQ], F32, name="qw_sb")
            qb_sb = cpool.tile([1, EQ], F32, name="qb_sb")
            kw_sb = cpool.tile([1, HD], F32, name="kw_sb")
            kb_sb = cpool.tile([1, HD], F32, name="kb_sb")
            nc.sync.dma_start(qw_sb[:], g.qw_d.ap())
            nc.sync.dma_start(qb_sb[:], g.qb_d.ap())
            nc.sync.dma_start(kw_sb[:], g.kw_d.ap())
            nc.sync.dma_start(kb_sb[:], g.kb_d.ap())
            g.qwB = cpool.tile([128, HQ, 2, 64], F32, name="qwB")
            g.qbB = cpool.tile([128, HQ, 2, 64], F32, name="qbB")
            g.kwB = cpool.tile([128, 2, 64], F32, name="kwB")
            g.kbB = cpool.tile([128, 2, 64], F32, name="kbB")

        # persistent activations
        g.xq_raw = persist.tile([128, NT, HQ, 2, 64], BF16, name="xq_raw")
        g.xk_raw = persist.tile([128, NT, 2, 64], BF16, name="xk_raw")
        g.v_s = persist.tile([128, NT, HD], BF16, name="v_s")
        g.stats_s = persist.tile([128, NT, 4], F32, name="stats_s")
        g.stats_g = persist.tile([128, NT, 4], F32, name="stats_g")
        g.qT_s = persist.tile([128, HQ, T], BF16, name="qT_s")
        g.kT_s = persist.tile([128, T], BF16, name="kT_s")

        g.mu_q = cpool.tile([128, NT], F32, name="mu_q")
        g.rstd_q = cpool.tile([128, NT], F32, name="rstd_q")
        g.nmr_q = cpool.tile([128, NT], F32, name="nmr_q")
        g.mu_k = cpool.tile([128, NT], F32, name="mu_k")
        g.rstd_k = cpool.tile([128, NT], F32, name="rstd_k")
        g.nmr_k = cpool.tile([128, NT], F32, name="nmr_k")
        g.tmp_a = cpool.tile([128, NT], F32, name="tmp_a")
        g.tmp_b = cpool.tile([128, NT], F32, name="tmp_b")

        def all_reduce_half(hb):
            # dmas + trigger all on gpsimd: it blocks there harmlessly (its
            # next consumer, attention's partition_broadcast, runs after AR1)
            ar_in = ardram.tile([128, ST, 4], F32, tag=f"ar_in{hb}")
            ar_out = ardram.tile([128, ST, 4], F32, tag=f"ar_out{hb}",
                                 addr_space="Shared")
            nc.gpsimd.dma_start(ar_in[:], g.stats_s[:, ts(hb, ST)])
            nc.gpsimd.collective_compute(
                "AllReduce", ALU.add,
                replica_groups=[list(range(NCORES))],
                ins=[ar_in.opt()], outs=[ar_out.opt()])
            nc.gpsimd.dma_start(g.stats_g[:, ts(hb, ST)], ar_out[:])

        # -------- phase 1: q/k/v projection + stats + AllReduces ----------
        with tc.tile_pool(name="p1w", bufs=1) as p1w, \
             tc.tile_pool(name="p1x", bufs=3) as p1x, \
             tc.tile_pool(name="p1s", bufs=2) as p1s, \
             tc.tile_pool(name="p2w", bufs=1) as p2w, \
             tc.tile_pool(name="ps1", bufs=1, space="PSUM") as ps1:
            g.p2 = p2w

            if not g.triv:
                for bcsrc, bcdst, wid in ((qw_sb, g.qwB, EQ),
                                          (qb_sb, g.qbB, EQ),
                                          (kw_sb, g.kwB, HD),
                                          (kb_sb, g.kbB, HD)):
                    ps_bc = ps1.tile([128, wid], F32, tag="psbc", bufs=2)
                    nc.tensor.matmul(ps_bc[:], lhsT=g.ones_r[:],
                                     rhs=bcsrc[:], start=True, stop=True)
                    nc.scalar.copy(flat2(bcdst[:]), ps_bc[:])

            def load_xpair(tp):  # 256-token pairs: 512B runs, full DMA rate
                x_t = p1x.tile([128, ND, 256], BF16, tag="x_t", bufs=2)
                for j8 in range(0, ND, 8):
                    nc.sync.dma_start(x_t[:, ds(j8, 8), :],
                                      g.xT_d.ap()[:, ds(j8, 8), ts(tp, 256)])
                return x_t

            wq_s = p1w.tile([128, ND, EQ], BF16, name="wq_s")
            wkv_s = p1w.tile([128, ND, 2 * HD], BF16, name="wkv_s")
            # tile-0 operand DMAs interleaved chunk-wise with its matmul
            # emission so the first MMs only wait on the first chunk set
            x_pre0 = p1x.tile([128, ND, 256], BF16, tag="x_t", bufs=2,
                              name="x_pre0")

            def chunk_dmas(j8):
                nc.sync.dma_start(x_pre0[:, ds(j8, 8), :],
                                  g.xT_d.ap()[:, ds(j8, 8), ts(0, 256)])
                nc.sync.dma_start(wq_s[:, ds(j8, 8), :],
                                  g.wqT_d.ap()[:, ds(j8, 8), :])
                nc.sync.dma_start(wkv_s[:, ds(j8, 8), :],
                                  g.wkvT_d.ap()[:, ds(j8, 8), :])

            chunk_dmas(0)
            chunk_dmas(8)

            def qkv_tile(ti, x_pre=None, interleave=None):
                if ti % 2 == 0:
                    g.x_cur = x_pre if x_pre is not None else load_xpair(
                        ti // 2)
                x_t = g.x_cur
                tsl = ts(ti % 2, 128)
                psq = ps1.tile([128, EQ], F32, tag="psq", bufs=3)
                pskv = ps1.tile([128, 2 * HD], F32, tag="pskv", bufs=3)
                # k|v fused into one 256-wide moving operand so each x-chunk
                # stationary is loaded once and reused by both streams
                for j in range(ND):
                    nc.tensor.matmul(psq[:], lhsT=x_t[:, j, tsl],
                                     rhs=wq_s[:, j, :],
                                     start=(j == 0), stop=(j == ND - 1))
                    nc.tensor.matmul(pskv[:], lhsT=x_t[:, j, tsl],
                                     rhs=wkv_s[:, j, :],
                                     start=(j == 0), stop=(j == ND - 1))
                    if interleave and j in interleave:
                        interleave[j]()
                psk = pskv[:, 0:HD]
                psv = pskv[:, HD:2 * HD]
                nc.scalar.copy(flat2(g.xq_raw[:, ti]), psq[:])
                nc.scalar.copy(flat2(g.xk_raw[:, ti]), psk)
                nc.scalar.copy(g.v_s[:, ti, :], psv)
                scrap = p1s.tile([128, EQ], BF16, tag="scrap", bufs=3)
                nc.vector.tensor_reduce(out=g.stats_s[:, ti, 0:1],
                                        in_=psq[:], axis=AX.X, op=ALU.add)
                nc.scalar.activation(scrap[:], psq[:], AF.Square,
                                     accum_out=g.stats_s[:, ti, 1:2])
                scrapk = p1s.tile([128, HD], BF16, tag="scrapk", bufs=3)
                nc.vector.tensor_reduce(out=g.stats_s[:, ti, 2:3],
                                        in_=psk, axis=AX.X, op=ALU.add)
                nc.scalar.activation(scrapk[:], psk, AF.Square,
                                     accum_out=g.stats_s[:, ti, 3:4])

            qkv_tile(0, x_pre=x_pre0,
                     interleave={7: lambda: chunk_dmas(16),
                                 15: lambda: chunk_dmas(24)})
            for ti in range(1, ST):
                qkv_tile(ti)
            all_reduce_half(0)      # lands while tiles 16..31 project
            for ti in range(ST, 24):
                qkv_tile(ti)
            _postamble(nc, g, 0)    # DVE-only wait on AR0; PE unaffected
            # weave batch-0 rope (ACT/DVE only — no PE transposes; psq/pskv
            # bufs=3 lets the PE run ~3 tiles ahead of an ACT queue that is
            # blocked on a late-landing AR0) into the phase-1 tail
            b0parts = [p for i in range(ST) for p in (("k", i), ("q", i))]
            for i, ti in enumerate(range(24, NT)):
                qkv_tile(ti)
                for wh, t2 in b0parts[i * 4:(i + 1) * 4]:
                    _ph2_rope(nc, g, t2, wh)
            all_reduce_half(1)      # lands during early attention b0

        # ------- phases 2..5: ph2(b0) | attn b0 | wo 0-15 + ph2(b1) |
        # -------               attn b1 | wo 16-31
        with tc.tile_pool(name="p34", bufs=1) as p34:
            g.oT_s = p34.tile([128, HQ, T], BF16, name="oT_s")
            g.woT_s = p34.tile([128, HQ, D], BF16, name="woT_s")
            nc.sync.dma_start(g.woT_s[:], g.woT_d.ap())

            # ---- pre-attention: batch-0 transposes (rope already done
            # ---- under the phase-1 tail); copies alternate DVE/ACT ----
            tp_order = ([("k", 0), ("k", 1)]
                        + [("q", i) for i in range(4)]
                        + [("k", i) for i in range(2, ST)]
                        + [("q", i) for i in range(4, ST)])
            with tc.tile_pool(name="tpa", bufs=1, space="PSUM") as tpp:
                g.tpp = tpp
                for n, (wh, t2) in enumerate(tp_order):
                    eng = nc.vector if n % 2 else nc.scalar
                    _ph2_tp(nc, g, t2, wh, ceng=eng)

            # ---- attention batch 0 (postamble(1) woven in) ----
            with tc.tile_pool(name="p3a", bufs=1) as p3, \
                 tc.tile_pool(name="ps3a", bufs=1, space="PSUM") as ps3:
                g.p3, g.ps3 = p3, ps3
                _attn_batch(nc, g, 0, {})
            # AR1 has certainly landed by now; first consumer is wo's fills
            _postamble(nc, g, 1)

            # ---- wo tiles 0-15, batch-1 ph2 rope as fills, transposes
            # ---- interleaved into the tail tiles ----
            b1_parts = ([("k", ti) for ti in range(ST, NT)]
                        + [("q", ti) for ti in range(ST, NT)])
            with tc.tile_pool(name="p2b", bufs=1) as p2b, \
                 tc.tile_pool(name="p4a", bufs=1) as p4, \
                 tc.tile_pool(name="ps4a", bufs=1, space="PSUM") as ps4:
                g.p2, g.p4, g.ps4, g.tpp = p2b, p4, ps4, ps4
                for ti in range(12):
                    ropes = b1_parts[ti * 8 // 3:(ti + 1) * 8 // 3]
                    _wo_tile(nc, g, ti)
                    for wh, t2 in ropes:
                        _ph2_rope(nc, g, t2, wh)
                for ti in range(12, ST):
                    tps = b1_parts[(ti - 12) * 8:(ti - 11) * 8]
                    _wo_tile(nc, g, ti, tp_parts=tps)

            # ---- attention batch 1 ----
            with tc.tile_pool(name="p3b", bufs=1) as p3, \
                 tc.tile_pool(name="ps3b", bufs=1, space="PSUM") as ps3:
                g.p3, g.ps3 = p3, ps3
                _attn_batch(nc, g, 1, {})

            # ---- wo tiles 16-31 ----
            with tc.tile_pool(name="p4b", bufs=1) as p4, \
                 tc.tile_pool(name="ps4b", bufs=1, space="PSUM") as ps4:
                g.p4, g.ps4 = p4, ps4
                for ti in range(ST, NT):
                    _wo_tile(nc, g, ti)


def _postamble(nc, g, hb):
    """mu/rstd/-mu*rstd for one AllReduce half (token tiles hb*ST..)."""
    sl = ts(hb, ST)

    def stat(k):
        return g.stats_g[:, sl, k:k + 1].rearrange("p t s -> p (t s)")

    for (mu_t, rstd_t, nmr_t, s0, s1, e_full) in (
            (g.mu_q, g.rstd_q, g.nmr_q, 0, 1, D),
            (g.mu_k, g.rstd_k, g.nmr_k, 2, 3, KV * HD)):
        nc.vector.tensor_scalar_mul(mu_t[:, sl], stat(s0), 1.0 / e_full)
        nc.vector.tensor_scalar_mul(g.tmp_a[:, sl], stat(s1), 1.0 / e_full)
        nc.vector.tensor_mul(g.tmp_b[:, sl], mu_t[:, sl], mu_t[:, sl])
        nc.vector.tensor_sub(g.tmp_a[:, sl], g.tmp_a[:, sl], g.tmp_b[:, sl])
        nc.scalar.activation(g.tmp_b[:, sl], g.tmp_a[:, sl], AF.Sqrt,
                             bias=g.eps_c[:])
        nc.vector.reciprocal(rstd_t[:, sl], g.tmp_b[:, sl])
        nc.vector.scalar_tensor_tensor(
            out=nmr_t[:, sl], in0=mu_t[:, sl], scalar=-1.0,
            in1=rstd_t[:, sl], op0=ALU.mult, op1=ALU.mult)


def _ph2_rope(nc, g, ti, which):
    """LN apply (on ACT) + de-interleaved rope (6 half-width DVE ops),
    written IN PLACE over the raw projection tile (dead after this)."""
    p2 = g.p2
    if which == "q":
        raw = g.xq_raw[:, ti]                 # [128, HQ, 2, 64]
        mu_t, rstd_t, nmr_t = g.mu_q, g.rstd_q, g.nmr_q
        wB = g.qwB if not g.triv else None
        bB = g.qbB if not g.triv else None
        nh = HQ
        cos_t = p2.tile([128, HQ, 1, 64], BF16, tag="cosq", bufs=2)
        sin_t = p2.tile([128, HQ, 1, 64], BF16, tag="sinq", bufs=2)
        # ACT-queue trigger: a WAR wait here must not block the sync
        # queue's x/weight prefetch stream
        nc.scalar.dma_start(cos_t[:].rearrange("p h o s -> p h (o s)"),
                            g.cosq_d.ap()[ts(ti, 128)])
        nc.scalar.dma_start(sin_t[:].rearrange("p h o s -> p h (o s)"),
                            g.sinq_d.ap()[ts(ti, 128)])
        xn_t = p2.tile([128, HQ, 2, 64], BF16, tag="xnq", bufs=2)
        mshape = [128, HQ, 1, 64]
        x0, x1 = xn_t[:, :, 0:1, :], xn_t[:, :, 1:2, :]
        rp0, rp1 = raw[:, :, 0:1, :], raw[:, :, 1:2, :]
    else:
        raw = g.xk_raw[:, ti]                 # [128, 2, 64]
        mu_t, rstd_t, nmr_t = g.mu_k, g.rstd_k, g.nmr_k
        wB = g.kwB if not g.triv else None
        bB = g.kbB if not g.triv else None
        nh = 1
        cos_t = p2.tile([128, 1, 64], BF16, tag="cosk", bufs=2)
        sin_t = p2.tile([128, 1, 64], BF16, tag="sink", bufs=2)
        nc.scalar.dma_start(flat2(cos_t[:]), g.cosk_d.ap()[ts(ti, 128)])
        nc.scalar.dma_start(flat2(sin_t[:]), g.sink_d.ap()[ts(ti, 128)])
        xn_t = p2.tile([128, 2, 64], BF16, tag="xnk", bufs=2)
        mshape = [128, 1, 64]
        x0, x1 = xn_t[:, 0:1, :], xn_t[:, 1:2, :]
        rp0, rp1 = raw[:, 0:1, :], raw[:, 1:2, :]

    # xn = (raw - mu) * rstd on ACT: scale=rstd, bias=-mu*rstd
    nc.scalar.activation(flat2(xn_t[:]), flat2(raw), AF.Identity,
                         bias=nmr_t[:, ti:ti + 1],
                         scale=rstd_t[:, ti:ti + 1])
    if wB is not None:
        nc.vector.tensor_mul(xn_t[:], xn_t[:], wB[:])
        nc.vector.tensor_add(xn_t[:], xn_t[:], bB[:])
    mA = p2.tile(mshape, BF16, tag=f"mA{which}", bufs=2)
    mB = p2.tile(mshape, BF16, tag=f"mB{which}", bufs=2)
    nc.vector.tensor_mul(mA[:], x0, cos_t[:])
    nc.vector.tensor_mul(mB[:], x1, sin_t[:])
    nc.vector.tensor_sub(rp0, mA[:], mB[:])
    mC = p2.tile(mshape, BF16, tag=f"mC{which}", bufs=2)
    mD = p2.tile(mshape, BF16, tag=f"mD{which}", bufs=2)
    nc.vector.tensor_mul(mC[:], x0, sin_t[:])
    nc.vector.tensor_mul(mD[:], x1, cos_t[:])
    nc.vector.tensor_add(rp1, mC[:], mD[:])


def _ph2_tp(nc, g, ti, which, ceng=None):
    """PE-transpose rope output [t,(hd)] -> [hd,t]; copies on ACT/DVE."""
    nh = HQ if which == "q" else 1
    for h in range(nh):
        src = g.xq_raw[:, ti, h] if which == "q" else g.xk_raw[:, ti]
        tp_ps = g.tpp.tile([128, 128], BF16, tag="tp", bufs=2)
        nc.tensor.transpose(tp_ps[:], flat2(src), g.ident[:])
        dst = (g.qT_s[:, h, ts(ti, 128)] if which == "q"
               else g.kT_s[:, ts(ti, 128)])
        if ceng is nc.vector:
            nc.vector.tensor_copy(dst, tp_ps[:])
        else:
            nc.scalar.copy(dst, tp_ps[:])


def _attn_batch(nc, g, b, fill):
    """Attention for one batch; head-paired kt loop. fill maps (qb, hp) ->
    "post2" emitted after that head-pair's epilogue."""
    p3, ps3 = g.p3, g.ps3
    for qb in range(NQB):
        for hp in range(2):
            h0, h1 = 2 * hp, 2 * hp + 1
            qsl = ds(b * S + qb * 512, 512)
            psVs = [ps3.tile([128, 512], F32, tag="psV", bufs=2,
                             name=f"psV{hi}") for hi in range(2)]

            def psb_mm(kt):
                t = ps3.tile([128, 2, 512], F32, tag="psB", bufs=3)
                for hi, h in enumerate((h0, h1)):
                    nc.tensor.matmul(
                        t[:, hi], lhsT=g.kT_s[:, ds(b * S + kt * 128, 128)],
                        rhs=g.qT_s[:, h, qsl], start=True, stop=True)
                return t

            psBs = [psb_mm(0)]
            L1s = ([], [])
            L2s = ([], [])
            L3s = ([], [])
            prevT = None
            for kt in range(ST):
                attnT = p3.tile([128, 2, 512], BF16, tag="attnT", bufs=4)
                nc.scalar.activation(flat2(attnT[:]), flat2(psBs[kt][:]),
                                     AF.Exp, bias=g.shift_c[:])
                if kt + 1 < ST:
                    psBs.append(psb_mm(kt + 1))
                for hi in range(2):
                    nc.tensor.matmul(psVs[hi][:],
                                     lhsT=g.v_s[:, b * ST + kt, :],
                                     rhs=attnT[:, hi], start=(kt == 0),
                                     stop=(kt == ST - 1))
                if kt % 2 == 1:
                    # bf16 add-tree for softmax denominators (kills 7 of the
                    # baseline's 8 ones-matmuls per qblock)
                    for hi in range(2):
                        L1 = p3.tile([128, 512], BF16, tag="L1", bufs=4)
                        nc.vector.tensor_add(L1[:], prevT[:, hi],
                                             attnT[:, hi])
                        L1s[hi].append(L1)
                        if len(L1s[hi]) % 2 == 0:
                            L2 = p3.tile([128, 512], BF16, tag="L2", bufs=3)
                            nc.vector.tensor_add(L2[:], L1s[hi][-2][:],
                                                 L1s[hi][-1][:])
                            L2s[hi].append(L2)
                            if len(L2s[hi]) % 2 == 0:
                                L3 = p3.tile([128, 512], BF16, tag="L3",
                                             bufs=4)
                                nc.vector.tensor_add(L3[:], L2s[hi][-2][:],
                                                     L2s[hi][-1][:])
                                L3s[hi].append(L3)
                prevT = attnT

            # epilogue: two accumulated [1,512] ones-matmuls per head into a
            # spare psB-tagged tile (rows 0/32 — no extra psum bank), then
            # bcast + reciprocal + scale. (gpsimd partition_all_reduce was
            # tried here: 3.5us/call sat in the psV-release chain and
            # stalled each head-pair boundary.)
            psSx = ps3.tile([128, 2, 512], F32, tag="psB", bufs=3,
                            name="psSx")
            for hi in range(2):
                for j, L3 in enumerate(L3s[hi]):
                    nc.tensor.matmul(psSx[32 * hi:32 * hi + 1, 0, :],
                                     lhsT=g.ones_c[:], rhs=L3[:],
                                     start=(j == 0), stop=(j == 1))
            for hi, h in enumerate((h0, h1)):
                sumR = p3.tile([1, 512], F32, tag="sumR", bufs=2)
                nc.vector.tensor_copy(sumR[:], psSx[32 * hi:32 * hi + 1, 0, :])
                bc_sb = p3.tile([128, 512], F32, tag="bc_sb", bufs=2)
                nc.gpsimd.partition_broadcast(bc_sb[:], sumR[:])
                rc_sb = p3.tile([128, 512], F32, tag="rc_sb", bufs=2)
                nc.vector.reciprocal_approx_fast(out=rc_sb[:], in_=bc_sb[:])
                nc.vector.tensor_mul(g.oT_s[:, h, qsl], psVs[hi][:],
                                     rc_sb[:])
            if fill.get((qb, hp)) == "post2":
                _postamble(nc, g, 1)


def _wo_tile(nc, g, ti, tp_parts=()):
    """Output projection for one 128-token tile (four 1024-wide quads,
    2-bank psO so transposes can share PSUM). tp_parts: batch-1 ph2
    transposes interleaved between quads to keep HAM warm."""
    p4, ps4 = g.p4, g.ps4
    tp_parts = list(tp_parts)
    for quad in range(4):
        psO = ps4.tile([128, 2, 512], F32, tag="psO", bufs=3)
        for h in range(HQ):
            for nb in range(2):
                nc.tensor.matmul(
                    psO[:, nb], lhsT=g.oT_s[:, h, ts(ti, 128)],
                    rhs=g.woT_s[:, h, ds(quad * 1024 + nb * 512, 512)],
                    start=(h == 0), stop=(h == HQ - 1))
        outst = p4.tile([128, 2, 512], BF16, tag="outst", bufs=3)
        if quad % 2 == 0:
            nc.vector.tensor_copy(flat2(outst[:]), flat2(psO[:]))
        else:
            nc.scalar.copy(flat2(outst[:]), flat2(psO[:]))
        nc.sync.dma_start(g.out_d.ap()[ts(ti, 128), ds(quad * 1024, 1024)],
                          flat2(outst[:]))
        for wh, t2 in tp_parts[quad * 2:quad * 2 + 2]:
            _ph2_tp(nc, g, t2, wh)


def _host_inputs(x, freqs_cis, wq, wk, wv, wo, q_norm_w, q_norm_b,
                 k_norm_w, k_norm_b, triv):
    bf = ml_dtypes.bfloat16
    f32 = np.float32
    x = np.asarray(x, f32)
    freqs_cis = np.asarray(freqs_cis, f32)
    wq = np.asarray(wq, f32)
    wk = np.asarray(wk, f32)
    wv = np.asarray(wv, f32)
    wo = np.asarray(wo, f32)
    q_norm_w = np.asarray(q_norm_w, f32)
    q_norm_b = np.asarray(q_norm_b, f32)
    k_norm_w = np.asarray(k_norm_w, f32)
    k_norm_b = np.asarray(k_norm_b, f32)

    xf = np.ascontiguousarray(x.reshape(T, D))
    xT_r = np.ascontiguousarray(
        xf.T.reshape(ND, 128, T).transpose(1, 0, 2)).astype(bf)

    # rope de-interleave: within each head, rows (even dims | odd dims)
    perm = np.concatenate([np.arange(0, HD, 2), np.arange(1, HD, 2)])
    scale = 1.0 / np.sqrt(np.float32(HD))
    cos = freqs_cis[:, :, 0]          # [S, 64]
    sin = freqs_cis[:, :, 1]
    cos2 = np.concatenate([cos] * B, 0)   # [T, 64]
    sin2 = np.concatenate([sin] * B, 0)
    cosq = np.ascontiguousarray(np.broadcast_to(
        (cos2 * scale)[:, None], (T, HQ, 64))).astype(bf)
    sinq = np.ascontiguousarray(np.broadcast_to(
        (sin2 * scale)[:, None], (T, HQ, 64))).astype(bf)
    cosk = np.ascontiguousarray(cos2).astype(bf)
    sink = np.ascontiguousarray(sin2).astype(bf)

    in_maps = []
    for c in range(NCORES):
        wq_c = wq[c * EQ:(c + 1) * EQ].reshape(HQ, HD, D)[:, perm].reshape(
            EQ, D)
        wk_c = wk[c * HD:(c + 1) * HD][perm]          # [128, D]
        wv_c = wv[c * HD:(c + 1) * HD]
        wo_c = wo[:, c * EQ:(c + 1) * EQ]             # [D, 512]
        wqT_r = np.ascontiguousarray(
            wq_c.T.reshape(ND, 128, EQ).transpose(1, 0, 2)).astype(bf)
        wkT_r = np.ascontiguousarray(
            wk_c.T.reshape(ND, 128, HD).transpose(1, 0, 2)).astype(bf)
        wvT_r = np.ascontiguousarray(
            wv_c.T.reshape(ND, 128, HD).transpose(1, 0, 2)).astype(bf)
        wkvT_r = np.ascontiguousarray(
            np.concatenate([wkT_r, wvT_r], axis=2))
        woT_r = np.ascontiguousarray(
            wo_c.T.reshape(HQ, 128, D).transpose(1, 0, 2)).astype(bf)
        im = {
            "xT": xT_r, "wqT": wqT_r, "wkvT": wkvT_r,
            "woT": woT_r, "cosq": cosq, "sinq": sinq, "cosk": cosk,
            "sink": sink,
        }
        if not triv:
            qp = np.concatenate([p + c * EQ for p in
                                 [h * HD + perm for h in range(HQ)]])
            im["qw"] = q_norm_w[qp].astype(f32).reshape(1, EQ)
            im["qb"] = q_norm_b[qp].astype(f32).reshape(1, EQ)
            im["kw"] = k_norm_w[c * HD + perm].astype(f32).reshape(1, HD)
            im["kb"] = k_norm_b[c * HD + perm].astype(f32).reshape(1, HD)
        in_maps.append(im)
    return in_maps


def _run_profiled(nc, in_maps):
    """bass2jax execute wrapped in an NRT profile capture; returns
    (results, max exec_time_ns across cores, trace_dir)."""
    import ctypes
    import glob
    import tempfile

    import jax
    from concourse import bass2jax
    import gauge.profiler
    from concourse.bass_utils import FishPath

    lib = ctypes.CDLL("/opt/axon/libaxon_pjrt.so")
    if not hasattr(lib, "axon_start_nrt_profile"):
        results = bass2jax.run_bass_via_pjrt(nc, in_maps, n_cores=NCORES)
        return results, None, None
    lib.axon_start_nrt_profile.argtypes = [ctypes.POINTER(ctypes.c_int64),
                                           ctypes.c_size_t]
    lib.axon_start_nrt_profile.restype = ctypes.c_int64
    lib.axon_stop_nrt_profile.argtypes = [ctypes.c_char_p]
    lib.axon_stop_nrt_profile.restype = ctypes.c_int64

    jax.devices()
    # warm-up execution: loads the NEFF and aligns core dispatch so the
    # profiled run isn't polluted by first-run start skew
    bass2jax.run_bass_via_pjrt(nc, in_maps, n_cores=NCORES)
    neff_dir = tempfile.mkdtemp(prefix="bassprof_")
    rc = lib.axon_start_nrt_profile(None, 0)
    if rc != 0:
        raise RuntimeError(f"axon_start_nrt_profile rc={rc}")
    try:
        results = bass2jax.run_bass_via_pjrt(nc, in_maps, n_cores=NCORES)
    finally:
        n = lib.axon_stop_nrt_profile(neff_dir.encode())
        print(f"profile: {n} ntff file(s) in {neff_dir}")
    ntffs = glob.glob(neff_dir + "/*_body*.ntff")
    if not ntffs:
        return results, None, None
    profile = gauge.profiler.Profile(
        profile_path=FishPath(neff_dir), kernel_dev_mode=True,
        profile_on_exit=False, bass_kernel=nc.m,
        offline_processing=True, fname="*_body*")
    exec_ns = None
    try:
        prs = profile.to_perfetto(model_index=list(range(NCORES)))
        times = [pr.exec_time_ns for pr in prs if pr.exec_time_ns]
        exec_ns = max(times) if times else None
    except Exception as e:  # profile parse best-effort
        print("profile parse failed:", e)
    return results, exec_ns, neff_dir


def kernel(x, freqs_cis, wq, wk, wv, wo, q_norm_w, q_norm_b,
           k_norm_w, k_norm_b):
    global LAST_EXEC_NS, LAST_TRACE_DIR
    triv = (np.allclose(np.asarray(q_norm_w), 1.0)
            and np.allclose(np.asarray(q_norm_b), 0.0)
            and np.allclose(np.asarray(k_norm_w), 1.0)
            and np.allclose(np.asarray(k_norm_b), 0.0))
    nc = _build(triv)
    in_maps = _host_inputs(x, freqs_cis, wq, wk, wv, wo,
                           q_norm_w, q_norm_b, k_norm_w, k_norm_b, triv)
    if PROFILE:
        results, LAST_EXEC_NS, LAST_TRACE_DIR = _run_profiled(nc, in_maps)
    else:
        res = bass_utils.run_bass_kernel_spmd(
            nc, in_maps, core_ids=list(range(NCORES)))
        results = res.results
        LAST_EXEC_NS = res.exec_time_ns
    acc = np.zeros((T, D), np.float32)
    for r in results:
        acc += np.asarray(r["out"], np.float32)
    return acc.reshape(B, S, D)


# revision 28
# speedup vs baseline: 1.0812x; 1.0577x over previous
"""GQA attention block (B=2,S=2048,D=4096,H=32,KV=8,HD=128) on 8 TRN2 NeuronCores.

Sharding: 8-way tensor parallel over heads. Core c owns kv-head c and q-heads
4c..4c+3 (wq/wk/wv column-sharded, wo row-sharded). The full-width Q/K
layernorms need cross-core mean/var, done with two tiny 32KB AllReduces.
Each core emits a partial [T,D] output; the host sums the 8 partials.

Schedule (v2 — restructured from the 1.15ms baseline):
  1. Projections for ALL 32 token tiles first; AR0 (tiles 0-15 stats) fires
     mid-phase and AR1 right after tile 31, so neither collective ever blocks
     the PE queue (the old layout wove AR-dependent transposes between the
     tail projection tiles and stalled the PE ~40us; AR1 also triggered ~90us
     late behind weave DVE work).
  2. Pre-attention block: LN postamble + rope + PE-transpose for all batch-0
     q/k tiles (DVE runs under the projection tail once AR0 lands).
     Rope operates on host-de-interleaved pairs (wq/wk rows permuted per head
     so (even,odd) components are contiguous) — 6 half-width DVE ops, no
     copies; q/k share the permutation so scores are unchanged. The LN apply
     runs on ACT (scale=rstd, bias=-mu*rstd per partition); the trivial
     affine (w=1,b=0, detected host-side) is skipped.
  3. Attention batch 0: head-PAIRED kt loop — each kT/v stationary load
     serves two heads' matmuls (the baseline paid ~4.3us/qblock of unhidden
     LDWEIGHTS); exp batched over both heads' score tiles ([128,2,512] PSUM,
     one ACT instr); softmax denominators via a bf16 add-tree on DVE ending
     in two accumulated [1,512] ones-matmuls into one PSUM bank (rows 0/32),
     replacing 8 ones-matmuls/qblock.
  4. wo projection for token tiles 0-15, with batch-1's rope as fills and its
     transposes interleaved into the last tiles (keeps HAM warm, tp shares
     PSUM with the 2-bank psO tiles).
  5. Attention batch 1 (same shape, no fills), then wo tiles 16-31.
All matmuls bf16 with f32 accumulation (fp8 DoubleRow measured 3.8e-2 rel
err per converted GEMM in simulation — over the 2e-2 budget, rejected).
"""

from contextlib import ExitStack

import numpy as np
import ml_dtypes

import concourse.bass as bass
import concourse.mybir as mybir
import concourse.tile as tile
from concourse import bacc
from concourse import bass_isa
from concourse import bass_utils
from concourse.bass import ts, ds
from concourse.masks import make_identity

BF16 = mybir.dt.bfloat16
F32 = mybir.dt.float32
AF = mybir.ActivationFunctionType
ALU = mybir.AluOpType
AX = mybir.AxisListType

B, S, D = 2, 2048, 4096
T = B * S                 # 4096 tokens
H, KV, HD = 32, 8, 128
NCORES = 8
HQ = H // NCORES          # 4 q heads per core
EQ = HQ * HD              # 512
NT = T // 128             # 32 token tiles
ND = D // 128             # 32 contraction chunks
ST = S // 128             # 16 seq tiles per batch
NQB = S // 512            # 4 q-blocks per (b,h)
EPS = 1e-5
SHIFT = 12.0              # constant softmax shift (scores verified < ~8)

PROFILE = False
LAST_EXEC_NS = None
LAST_TRACE_DIR = None
_CACHE = {}


def flat2(ap):  # flatten all free dims -> [P, prod(free)]
    n = len(ap.shape)
    if n == 2:
        return ap
    names = " ".join(f"d{i}" for i in range(n - 1))
    return ap.rearrange(f"p {names} -> p ({names})")


class _Ctx:
    pass


def _build(triv):
    key = ("nc", triv)
    if key in _CACHE:
        return _CACHE[key]
    nc = bacc.Bacc("TRN2", target_bir_lowering=False, debug=False,
                   num_devices=NCORES)

    g = _Ctx()
    g.triv = triv
    g.xT_d = nc.dram_tensor("xT", [128, ND, T], BF16, kind="ExternalInput")
    g.wqT_d = nc.dram_tensor("wqT", [128, ND, EQ], BF16, kind="ExternalInput")
    g.wkvT_d = nc.dram_tensor("wkvT", [128, ND, 2 * HD], BF16,
                              kind="ExternalInput")
    g.woT_d = nc.dram_tensor("woT", [128, HQ, D], BF16, kind="ExternalInput")
    g.cosq_d = nc.dram_tensor("cosq", [T, HQ, 64], BF16, kind="ExternalInput")
    g.sinq_d = nc.dram_tensor("sinq", [T, HQ, 64], BF16, kind="ExternalInput")
    g.cosk_d = nc.dram_tensor("cosk", [T, 64], BF16, kind="ExternalInput")
    g.sink_d = nc.dram_tensor("sink", [T, 64], BF16, kind="ExternalInput")
    if not triv:
        g.qw_d = nc.dram_tensor("qw", [1, EQ], F32, kind="ExternalInput")
        g.qb_d = nc.dram_tensor("qb", [1, EQ], F32, kind="ExternalInput")
        g.kw_d = nc.dram_tensor("kw", [1, HD], F32, kind="ExternalInput")
        g.kb_d = nc.dram_tensor("kb", [1, HD], F32, kind="ExternalInput")
    g.out_d = nc.dram_tensor("out", [T, D], BF16, kind="ExternalOutput")

    with tile.TileContext(nc) as tc:
        _emit(nc, tc, g)
    nc.compile()
    _CACHE[key] = nc
    return nc


def _emit(nc, tc, g):
    ctx = ExitStack()
    with ctx:
        cpool = ctx.enter_context(tc.tile_pool(name="cpool", bufs=1))
        persist = ctx.enter_context(tc.tile_pool(name="persist", bufs=1))
        ardram = ctx.enter_context(
            tc.tile_pool(name="ardram", bufs=1, space="DRAM"))

        # ---- constants ----
        g.ident = cpool.tile([128, 128], BF16, name="ident")
        make_identity(nc, g.ident[:])
        g.ones_r = cpool.tile([1, 128], F32, name="ones_r")   # K=1 bcast lhsT
        nc.vector.memset(g.ones_r[:], 1.0)
        g.ones_c = cpool.tile([128, 1], BF16, name="ones_c")  # psum-col lhsT
        nc.vector.memset(g.ones_c[:], 1.0)
        g.eps_c = cpool.tile([128, 1], F32, name="eps_c")
        nc.vector.memset(g.eps_c[:], EPS)
        g.shift_c = cpool.tile([128, 1], F32, name="shift_c")
        nc.vector.memset(g.shift_c[:], -SHIFT)

        if not g.triv:
            qw_sb = cpool.tile([1, EQ], F32, name="qw_sb")
            qb_sb = cpool.tile([1, EQ], F32, name="qb_sb")
            kw_sb = cpool.tile([1, HD], F32, name="kw_sb")
            kb_sb = cpool.tile([1, HD], F32, name="kb_sb")
            nc.sync.dma_start(qw_sb[:], g.qw_d.ap())
            nc.sync.dma_start(qb_sb[:], g.qb_d.ap())
            nc.sync.dma_start(kw_sb[:], g.kw_d.ap())
            nc.sync.dma_start(kb_sb[:], g.kb_d.ap())
            g.qwB = cpool.tile([128, HQ, 2, 64], F32, name="qwB")
            g.qbB = cpool.tile([128, HQ, 2, 64], F32, name="qbB")
            g.kwB = cpool.tile([128, 2, 64], F32, name="kwB")
            g.kbB = cpool.tile([128, 2, 64], F32, name="kbB")

        # persistent activations
        g.xq_raw = persist.tile([128, NT, HQ, 2, 64], BF16, name="xq_raw")
        g.xk_raw = persist.tile([128, NT, 2, 64], BF16, name="xk_raw")
        g.v_s = persist.tile([128, NT, HD], BF16, name="v_s")
        g.stats_s = persist.tile([128, NT, 4], F32, name="stats_s")
        g.stats_g = persist.tile([128, NT, 4], F32, name="stats_g")
        g.qT_s = persist.tile([128, HQ, T], BF16, name="qT_s")
        g.kT_s = persist.tile([128, T], BF16, name="kT_s")

        g.mu_q = cpool.tile([128, NT], F32, name="mu_q")
        g.rstd_q = cpool.tile([128, NT], F32, name="rstd_q")
        g.mu_k = cpool.tile([128, NT], F32, name="mu_k")
        g.rstd_k = cpool.tile([128, NT], F32, name="rstd_k")
        g.tmp_a = cpool.tile([128, NT], F32, name="tmp_a")
        g.tmp_b = cpool.tile([128, NT], F32, name="tmp_b")

        g.ar_out = {}

        def all_reduce_start(hb):
            ar_in = ardram.tile([128, ST, 4], F32, tag=f"ar_in{hb}")
            ar_out = ardram.tile([128, ST, 4], F32, tag=f"ar_out{hb}",
                                 addr_space="Shared")
            g.ar_out[hb] = ar_out
            nc.gpsimd.dma_start(ar_in[:], g.stats_s[:, ts(hb, ST)])
            nc.gpsimd.collective_compute(
                "AllReduce", ALU.add,
                replica_groups=[list(range(NCORES))],
                ins=[ar_in.opt()], outs=[ar_out.opt()])

        def statsg_fetch(hb, eng):
            # the fetch BLOCKS its engine until the collective lands — put it
            # on an engine/queue position where that wait is provably free
            eng.dma_start(g.stats_g[:, ts(hb, ST)], g.ar_out[hb][:])

        # -------- phase 1: q/k/v projection + stats + AllReduces ----------
        with tc.tile_pool(name="p1w", bufs=1) as p1w, \
             tc.tile_pool(name="p1x", bufs=3) as p1x, \
             tc.tile_pool(name="p1s", bufs=2) as p1s, \
             tc.tile_pool(name="p2w", bufs=1) as p2w, \
             tc.tile_pool(name="ps1", bufs=1, space="PSUM") as ps1:
            g.p2 = p2w

            if not g.triv:
                for bcsrc, bcdst, wid in ((qw_sb, g.qwB, EQ),
                                          (qb_sb, g.qbB, EQ),
                                          (kw_sb, g.kwB, HD),
                                          (kb_sb, g.kbB, HD)):
                    ps_bc = ps1.tile([128, wid], F32, tag="psbc", bufs=2)
                    nc.tensor.matmul(ps_bc[:], lhsT=g.ones_r[:],
                                     rhs=bcsrc[:], start=True, stop=True)
                    nc.scalar.copy(flat2(bcdst[:]), ps_bc[:])

            def load_xpair(tp):  # 256-token pairs: 512B runs, full DMA rate
                x_t = p1x.tile([128, ND, 256], BF16, tag="x_t", bufs=2)
                for j8 in range(0, ND, 8):
                    nc.sync.dma_start(x_t[:, ds(j8, 8), :],
                                      g.xT_d.ap()[:, ds(j8, 8), ts(tp, 256)])
                return x_t

            wq_s = p1w.tile([128, ND, EQ], BF16, name="wq_s")
            wkv_s = p1w.tile([128, ND, 2 * HD], BF16, name="wkv_s")
            # tile-0 operand DMAs interleaved chunk-wise with its matmul
            # emission so the first MMs only wait on the first chunk set
            x_pre0 = p1x.tile([128, ND, 256], BF16, tag="x_t", bufs=2,
                              name="x_pre0")

            def chunk_dmas(j8):
                nc.sync.dma_start(x_pre0[:, ds(j8, 8), :],
                                  g.xT_d.ap()[:, ds(j8, 8), ts(0, 256)])
                nc.sync.dma_start(wq_s[:, ds(j8, 8), :],
                                  g.wqT_d.ap()[:, ds(j8, 8), :])
                nc.sync.dma_start(wkv_s[:, ds(j8, 8), :],
                                  g.wkvT_d.ap()[:, ds(j8, 8), :])

            chunk_dmas(0)
            chunk_dmas(8)

            def qkv_tile(ti, x_pre=None, interleave=None):
                if ti % 2 == 0:
                    g.x_cur = x_pre if x_pre is not None else load_xpair(
                        ti // 2)
                x_t = g.x_cur
                tsl = ts(ti % 2, 128)
                psq = ps1.tile([128, EQ], F32, tag="psq", bufs=3)
                pskv = ps1.tile([128, 2 * HD], F32, tag="pskv", bufs=3)
                # k|v fused into one 256-wide moving operand so each x-chunk
                # stationary is loaded once and reused by both streams
                for j in range(ND):
                    nc.tensor.matmul(psq[:], lhsT=x_t[:, j, tsl],
                                     rhs=wq_s[:, j, :],
                                     start=(j == 0), stop=(j == ND - 1))
                    nc.tensor.matmul(pskv[:], lhsT=x_t[:, j, tsl],
                                     rhs=wkv_s[:, j, :],
                                     start=(j == 0), stop=(j == ND - 1))
                    if interleave and j in interleave:
                        interleave[j]()
                psk = pskv[:, 0:HD]
                psv = pskv[:, HD:2 * HD]
                nc.scalar.copy(flat2(g.xq_raw[:, ti]), psq[:])
                nc.scalar.copy(flat2(g.xk_raw[:, ti]), psk)
                nc.scalar.copy(g.v_s[:, ti, :], psv)
                # stats fully on ACT so the DVE queue carries ONLY AR-gated
                # work in phase 1 (a late AllReduce then never blocks the
                # psum-release chain that paces the PE)
                scrap = p1s.tile([128, EQ], BF16, tag="scrap", bufs=3)
                nc.scalar.activation(scrap[:], psq[:], AF.Identity,
                                     accum_out=g.stats_s[:, ti, 0:1])
                scrap2 = p1s.tile([128, EQ], BF16, tag="scrap2", bufs=3)
                nc.scalar.activation(scrap2[:], psq[:], AF.Square,
                                     accum_out=g.stats_s[:, ti, 1:2])
                scrapk = p1s.tile([128, HD], BF16, tag="scrapk", bufs=3)
                nc.scalar.activation(scrapk[:], psk, AF.Identity,
                                     accum_out=g.stats_s[:, ti, 2:3])
                scrapk2 = p1s.tile([128, HD], BF16, tag="scrapk2", bufs=3)
                nc.scalar.activation(scrapk2[:], psk, AF.Square,
                                     accum_out=g.stats_s[:, ti, 3:4])

            qkv_tile(0, x_pre=x_pre0,
                     interleave={7: lambda: chunk_dmas(16),
                                 15: lambda: chunk_dmas(24)})
            for ti in range(1, ST):
                qkv_tile(ti)
            all_reduce_start(0)     # lands while tiles 16..31 project
            statsg_fetch(0, nc.gpsimd)
            _postamble(nc, g, 0)    # DVE waits AR0; DVE gates nothing else
            # weave batch-0 rope (pure DVE + gpsimd cos DMAs; ACT and sync
            # stay AR-free, so a late AllReduce can never reach the PE's
            # psum-release chain)
            b0parts = [p for i in range(ST) for p in (("k", i), ("q", i))]
            for i, ti in enumerate(range(ST, NT)):
                qkv_tile(ti)
                for wh, t2 in b0parts[i * 2:(i + 1) * 2]:
                    _ph2_rope(nc, g, t2, wh)
            all_reduce_start(1)     # lands during early attention b0

        # ------- phases 2..5: ph2(b0) | attn b0 | wo 0-15 + ph2(b1) |
        # -------               attn b1 | wo 16-31
        with tc.tile_pool(name="p34", bufs=1) as p34:
            g.oT_s = p34.tile([128, HQ, T], BF16, name="oT_s")
            g.woT_s = p34.tile([128, HQ, D], BF16, name="woT_s")
            nc.sync.dma_start(g.woT_s[:], g.woT_d.ap())

            # ---- pre-attention: batch-0 transposes (rope already done
            # ---- under the phase-1 tail); copies alternate DVE/ACT ----
            tp_order = ([("k", 0), ("k", 1)]
                        + [("q", i) for i in range(4)]
                        + [("k", i) for i in range(2, ST)]
                        + [("q", i) for i in range(4, ST)])
            with tc.tile_pool(name="tpa", bufs=1, space="PSUM") as tpp:
                g.tpp = tpp
                for n, (wh, t2) in enumerate(tp_order):
                    eng = nc.vector if n % 2 else nc.scalar
                    _ph2_tp(nc, g, t2, wh, ceng=eng)

            # ---- attention batch 0 (postamble(1) woven in) ----
            with tc.tile_pool(name="p3a", bufs=1) as p3, \
                 tc.tile_pool(name="ps3a", bufs=1, space="PSUM") as ps3:
                g.p3, g.ps3 = p3, ps3
                _attn_batch(nc, g, 0, {})
            # AR1 landed during b0; fetching here costs ACT/DVE nothing and
            # keeps the gpsimd broadcast stream unblocked during b0
            statsg_fetch(1, nc.scalar)
            _postamble(nc, g, 1)

            # ---- wo tiles 0-15, batch-1 ph2 rope as fills, transposes
            # ---- interleaved into the tail tiles ----
            b1_parts = ([("k", ti) for ti in range(ST, NT)]
                        + [("q", ti) for ti in range(ST, NT)])
            with tc.tile_pool(name="p2b", bufs=1) as p2b, \
                 tc.tile_pool(name="p4a", bufs=1) as p4, \
                 tc.tile_pool(name="ps4a", bufs=1, space="PSUM") as ps4:
                g.p2, g.p4, g.ps4, g.tpp = p2b, p4, ps4, ps4
                for ti in range(12):
                    ropes = b1_parts[ti * 8 // 3:(ti + 1) * 8 // 3]
                    _wo_tile(nc, g, ti)
                    for wh, t2 in ropes:
                        _ph2_rope(nc, g, t2, wh)
                for ti in range(12, ST):
                    tps = b1_parts[(ti - 12) * 8:(ti - 11) * 8]
                    _wo_tile(nc, g, ti, tp_parts=tps)

            # ---- attention batch 1 ----
            with tc.tile_pool(name="p3b", bufs=1) as p3, \
                 tc.tile_pool(name="ps3b", bufs=1, space="PSUM") as ps3:
                g.p3, g.ps3 = p3, ps3
                _attn_batch(nc, g, 1, {})

            # ---- wo tiles 16-31 ----
            with tc.tile_pool(name="p4b", bufs=1) as p4, \
                 tc.tile_pool(name="ps4b", bufs=1, space="PSUM") as ps4:
                g.p4, g.ps4 = p4, ps4
                for ti in range(ST, NT):
                    _wo_tile(nc, g, ti)


def _postamble(nc, g, hb):
    """mu/rstd/-mu*rstd for one AllReduce half (token tiles hb*ST..)."""
    sl = ts(hb, ST)

    def stat(k):
        return g.stats_g[:, sl, k:k + 1].rearrange("p t s -> p (t s)")

    for (mu_t, rstd_t, s0, s1, e_full) in (
            (g.mu_q, g.rstd_q, 0, 1, D),
            (g.mu_k, g.rstd_k, 2, 3, KV * HD)):
        nc.vector.tensor_scalar_mul(mu_t[:, sl], stat(s0), 1.0 / e_full)
        nc.vector.tensor_scalar_mul(g.tmp_a[:, sl], stat(s1), 1.0 / e_full)
        nc.vector.tensor_mul(g.tmp_b[:, sl], mu_t[:, sl], mu_t[:, sl])
        nc.vector.tensor_sub(g.tmp_a[:, sl], g.tmp_a[:, sl], g.tmp_b[:, sl])
        nc.scalar.activation(g.tmp_b[:, sl], g.tmp_a[:, sl], AF.Sqrt,
                             bias=g.eps_c[:])
        nc.vector.reciprocal(rstd_t[:, sl], g.tmp_b[:, sl])


def _ph2_rope(nc, g, ti, which):
    """LN apply (on ACT) + de-interleaved rope (6 half-width DVE ops),
    written IN PLACE over the raw projection tile (dead after this)."""
    p2 = g.p2
    if which == "q":
        raw = g.xq_raw[:, ti]                 # [128, HQ, 2, 64]
        mu_t, rstd_t = g.mu_q, g.rstd_q
        wB = g.qwB if not g.triv else None
        bB = g.qbB if not g.triv else None
        nh = HQ
        cos_t = p2.tile([128, HQ, 1, 64], BF16, tag="cosq", bufs=4)
        sin_t = p2.tile([128, HQ, 1, 64], BF16, tag="sinq", bufs=4)
        # gpsimd-queue trigger: a WAR wait here (rope is AR-gated) must not
        # block the ACT copy stream or the sync x/weight prefetch stream
        nc.gpsimd.dma_start(cos_t[:].rearrange("p h o s -> p h (o s)"),
                            g.cosq_d.ap()[ts(ti, 128)])
        nc.gpsimd.dma_start(sin_t[:].rearrange("p h o s -> p h (o s)"),
                            g.sinq_d.ap()[ts(ti, 128)])
        xn_t = p2.tile([128, HQ, 2, 64], BF16, tag="xnq", bufs=2)
        mshape = [128, HQ, 1, 64]
        x0, x1 = xn_t[:, :, 0:1, :], xn_t[:, :, 1:2, :]
        rp0, rp1 = raw[:, :, 0:1, :], raw[:, :, 1:2, :]
    else:
        raw = g.xk_raw[:, ti]                 # [128, 2, 64]
        mu_t, rstd_t = g.mu_k, g.rstd_k
        wB = g.kwB if not g.triv else None
        bB = g.kbB if not g.triv else None
        nh = 1
        cos_t = p2.tile([128, 1, 64], BF16, tag="cosk", bufs=4)
        sin_t = p2.tile([128, 1, 64], BF16, tag="sink", bufs=4)
        nc.gpsimd.dma_start(flat2(cos_t[:]), g.cosk_d.ap()[ts(ti, 128)])
        nc.gpsimd.dma_start(flat2(sin_t[:]), g.sink_d.ap()[ts(ti, 128)])
        xn_t = p2.tile([128, 2, 64], BF16, tag="xnk", bufs=2)
        mshape = [128, 1, 64]
        x0, x1 = xn_t[:, 0:1, :], xn_t[:, 1:2, :]
        rp0, rp1 = raw[:, 0:1, :], raw[:, 1:2, :]

    # xn = (raw - mu) * rstd, fused on DVE (keeps ACT AR-free)
    nc.vector.tensor_scalar(out=flat2(xn_t[:]), in0=flat2(raw),
                            scalar1=mu_t[:, ti:ti + 1],
                            scalar2=rstd_t[:, ti:ti + 1],
                            op0=ALU.subtract, op1=ALU.mult)
    if wB is not None:
        nc.vector.tensor_mul(xn_t[:], xn_t[:], wB[:])
        nc.vector.tensor_add(xn_t[:], xn_t[:], bB[:])
    mA = p2.tile(mshape, BF16, tag=f"mA{which}", bufs=2)
    mB = p2.tile(mshape, BF16, tag=f"mB{which}", bufs=2)
    nc.vector.tensor_mul(mA[:], x0, cos_t[:])
    nc.vector.tensor_mul(mB[:], x1, sin_t[:])
    nc.vector.tensor_sub(rp0, mA[:], mB[:])
    mC = p2.tile(mshape, BF16, tag=f"mC{which}", bufs=2)
    mD = p2.tile(mshape, BF16, tag=f"mD{which}", bufs=2)
    nc.vector.tensor_mul(mC[:], x0, sin_t[:])
    nc.vector.tensor_mul(mD[:], x1, cos_t[:])
    nc.vector.tensor_add(rp1, mC[:], mD[:])


def _ph2_tp(nc, g, ti, which, ceng=None):
    """PE-transpose rope output [t,(hd)] -> [hd,t]; copies on ACT/DVE."""
    nh = HQ if which == "q" else 1
    for h in range(nh):
        src = g.xq_raw[:, ti, h] if which == "q" else g.xk_raw[:, ti]
        tp_ps = g.tpp.tile([128, 128], BF16, tag="tp", bufs=2)
        nc.tensor.transpose(tp_ps[:], flat2(src), g.ident[:])
        dst = (g.qT_s[:, h, ts(ti, 128)] if which == "q"
               else g.kT_s[:, ts(ti, 128)])
        if ceng is nc.vector:
            nc.vector.tensor_copy(dst, tp_ps[:])
        else:
            nc.scalar.copy(dst, tp_ps[:])


def _attn_batch(nc, g, b, fill):
    """Attention for one batch; head-paired kt loop. fill maps (qb, hp) ->
    "post2" emitted after that head-pair's epilogue."""
    p3, ps3 = g.p3, g.ps3
    pending = [None]
    for qb in range(NQB):
        for hp in range(2):
            h0, h1 = 2 * hp, 2 * hp + 1
            qsl = ds(b * S + qb * 512, 512)
            psVs = [ps3.tile([128, 512], F32, tag="psV", bufs=2,
                             name=f"psV{hi}") for hi in range(2)]

            def psb_mm(kt):
                t = ps3.tile([128, 2, 512], F32, tag="psB", bufs=3)
                for hi, h in enumerate((h0, h1)):
                    nc.tensor.matmul(
                        t[:, hi], lhsT=g.kT_s[:, ds(b * S + kt * 128, 128)],
                        rhs=g.qT_s[:, h, qsl], start=True, stop=True)
                return t

            psBs = [psb_mm(0)]
            # previous iteration's epilogue emits AFTER this iteration's
            # first score matmuls so the PE never idles on the DVE tree tail
            if pending[0] is not None:
                pending[0]()
                pending[0] = None
            L1s = ([], [])
            L2s = ([], [])
            L3s = ([], [])
            prevT = None
            for kt in range(ST):
                attnT = p3.tile([128, 2, 512], BF16, tag="attnT", bufs=4)
                nc.scalar.activation(flat2(attnT[:]), flat2(psBs[kt][:]),
                                     AF.Exp, bias=g.shift_c[:])
                if kt + 1 < ST:
                    psBs.append(psb_mm(kt + 1))
                for hi in range(2):
                    nc.tensor.matmul(psVs[hi][:],
                                     lhsT=g.v_s[:, b * ST + kt, :],
                                     rhs=attnT[:, hi], start=(kt == 0),
                                     stop=(kt == ST - 1))
                if kt % 2 == 1:
                    # bf16 add-tree for softmax denominators (kills 7 of the
                    # baseline's 8 ones-matmuls per qblock)
                    for hi in range(2):
                        L1 = p3.tile([128, 512], BF16, tag="L1", bufs=4)
                        nc.vector.tensor_add(L1[:], prevT[:, hi],
                                             attnT[:, hi])
                        L1s[hi].append(L1)
                        if len(L1s[hi]) % 2 == 0:
                            L2 = p3.tile([128, 512], BF16, tag="L2", bufs=3)
                            nc.vector.tensor_add(L2[:], L1s[hi][-2][:],
                                                 L1s[hi][-1][:])
                            L2s[hi].append(L2)
                            if len(L2s[hi]) % 2 == 0:
                                L3 = p3.tile([128, 512], BF16, tag="L3",
                                             bufs=4)
                                nc.vector.tensor_add(L3[:], L2s[hi][-2][:],
                                                     L2s[hi][-1][:])
                                L3s[hi].append(L3)
                prevT = attnT

            # epilogue (deferred): two accumulated [1,512] ones-matmuls
            # per head into a spare psB-tagged tile (rows 0/32 — no extra
            # psum bank), then bcast + reciprocal + scale. (gpsimd
            # partition_all_reduce was tried: 3.5us/call sat in the
            # psV-release chain and stalled each head-pair boundary.)
            def epilogue(L3s=L3s, psVs=psVs, h0=h0, h1=h1, qsl=qsl):
                psSx = ps3.tile([128, 2, 512], F32, tag="psB", bufs=3,
                                name="psSx")
                for hi in range(2):
                    for j, L3 in enumerate(L3s[hi]):
                        nc.tensor.matmul(psSx[32 * hi:32 * hi + 1, 0, :],
                                         lhsT=g.ones_c[:], rhs=L3[:],
                                         start=(j == 0), stop=(j == 1))
                for hi, h in enumerate((h0, h1)):
                    sumR = p3.tile([1, 512], F32, tag="sumR", bufs=2)
                    nc.vector.tensor_copy(sumR[:],
                                          psSx[32 * hi:32 * hi + 1, 0, :])
                    bc_sb = p3.tile([128, 512], F32, tag="bc_sb", bufs=2)
                    nc.gpsimd.partition_broadcast(bc_sb[:], sumR[:])
                    rc_sb = p3.tile([128, 512], F32, tag="rc_sb", bufs=2)
                    nc.vector.reciprocal_approx_fast(out=rc_sb[:],
                                                     in_=bc_sb[:])
                    nc.vector.tensor_mul(g.oT_s[:, h, qsl], psVs[hi][:],
                                         rc_sb[:])

            pending[0] = epilogue
    if pending[0] is not None:
        pending[0]()
        pending[0] = None


def _wo_tile(nc, g, ti, tp_parts=()):
    """Output projection for one 128-token tile (four 1024-wide quads,
    2-bank psO so transposes can share PSUM). tp_parts: batch-1 ph2
    transposes interleaved between quads to keep HAM warm."""
    p4, ps4 = g.p4, g.ps4
    tp_parts = list(tp_parts)
    for quad in range(4):
        psO = ps4.tile([128, 2, 512], F32, tag="psO", bufs=3)
        for h in range(HQ):
            for nb in range(2):
                nc.tensor.matmul(
                    psO[:, nb], lhsT=g.oT_s[:, h, ts(ti, 128)],
                    rhs=g.woT_s[:, h, ds(quad * 1024 + nb * 512, 512)],
                    start=(h == 0), stop=(h == HQ - 1))
        outst = p4.tile([128, 2, 512], BF16, tag="outst", bufs=3)
        if quad % 2 == 0:
            nc.vector.tensor_copy(flat2(outst[:]), flat2(psO[:]))
        else:
            nc.scalar.copy(flat2(outst[:]), flat2(psO[:]))
        nc.sync.dma_start(g.out_d.ap()[ts(ti, 128), ds(quad * 1024, 1024)],
                          flat2(outst[:]))
        for n2, (wh, t2) in enumerate(tp_parts[quad * 2:quad * 2 + 2]):
            _ph2_tp(nc, g, t2, wh,
                    ceng=nc.vector if (quad + n2) % 2 else nc.scalar)


def _host_inputs(x, freqs_cis, wq, wk, wv, wo, q_norm_w, q_norm_b,
                 k_norm_w, k_norm_b, triv):
    bf = ml_dtypes.bfloat16
    f32 = np.float32
    x = np.asarray(x, f32)
    freqs_cis = np.asarray(freqs_cis, f32)
    wq = np.asarray(wq, f32)
    wk = np.asarray(wk, f32)
    wv = np.asarray(wv, f32)
    wo = np.asarray(wo, f32)
    q_norm_w = np.asarray(q_norm_w, f32)
    q_norm_b = np.asarray(q_norm_b, f32)
    k_norm_w = np.asarray(k_norm_w, f32)
    k_norm_b = np.asarray(k_norm_b, f32)

    xf = np.ascontiguousarray(x.reshape(T, D))
    xT_r = np.ascontiguousarray(
        xf.T.reshape(ND, 128, T).transpose(1, 0, 2)).astype(bf)

    # rope de-interleave: within each head, rows (even dims | odd dims)
    perm = np.concatenate([np.arange(0, HD, 2), np.arange(1, HD, 2)])
    scale = 1.0 / np.sqrt(np.float32(HD))
    cos = freqs_cis[:, :, 0]          # [S, 64]
    sin = freqs_cis[:, :, 1]
    cos2 = np.concatenate([cos] * B, 0)   # [T, 64]
    sin2 = np.concatenate([sin] * B, 0)
    cosq = np.ascontiguousarray(np.broadcast_to(
        (cos2 * scale)[:, None], (T, HQ, 64))).astype(bf)
    sinq = np.ascontiguousarray(np.broadcast_to(
        (sin2 * scale)[:, None], (T, HQ, 64))).astype(bf)
    cosk = np.ascontiguousarray(cos2).astype(bf)
    sink = np.ascontiguousarray(sin2).astype(bf)

    in_maps = []
    for c in range(NCORES):
        wq_c = wq[c * EQ:(c + 1) * EQ].reshape(HQ, HD, D)[:, perm].reshape(
            EQ, D)
        wk_c = wk[c * HD:(c + 1) * HD][perm]          # [128, D]
        wv_c = wv[c * HD:(c + 1) * HD]
        wo_c = wo[:, c * EQ:(c + 1) * EQ]             # [D, 512]
        wqT_r = np.ascontiguousarray(
            wq_c.T.reshape(ND, 128, EQ).transpose(1, 0, 2)).astype(bf)
        wkT_r = np.ascontiguousarray(
            wk_c.T.reshape(ND, 128, HD).transpose(1, 0, 2)).astype(bf)
        wvT_r = np.ascontiguousarray(
            wv_c.T.reshape(ND, 128, HD).transpose(1, 0, 2)).astype(bf)
        wkvT_r = np.ascontiguousarray(
            np.concatenate([wkT_r, wvT_r], axis=2))
        woT_r = np.ascontiguousarray(
            wo_c.T.reshape(HQ, 128, D).transpose(1, 0, 2)).astype(bf)
        im = {
            "xT": xT_r, "wqT": wqT_r, "wkvT": wkvT_r,
            "woT": woT_r, "cosq": cosq, "sinq": sinq, "cosk": cosk,
            "sink": sink,
        }
        if not triv:
            qp = np.concatenate([p + c * EQ for p in
                                 [h * HD + perm for h in range(HQ)]])
            im["qw"] = q_norm_w[qp].astype(f32).reshape(1, EQ)
            im["qb"] = q_norm_b[qp].astype(f32).reshape(1, EQ)
            im["kw"] = k_norm_w[c * HD + perm].astype(f32).reshape(1, HD)
            im["kb"] = k_norm_b[c * HD + perm].astype(f32).reshape(1, HD)
        in_maps.append(im)
    return in_maps


def _run_profiled(nc, in_maps):
    """bass2jax execute wrapped in an NRT profile capture; returns
    (results, max exec_time_ns across cores, trace_dir)."""
    import ctypes
    import glob
    import tempfile

    import jax
    from concourse import bass2jax
    import gauge.profiler
    from concourse.bass_utils import FishPath

    lib = ctypes.CDLL("/opt/axon/libaxon_pjrt.so")
    if not hasattr(lib, "axon_start_nrt_profile"):
        results = bass2jax.run_bass_via_pjrt(nc, in_maps, n_cores=NCORES)
        return results, None, None
    lib.axon_start_nrt_profile.argtypes = [ctypes.POINTER(ctypes.c_int64),
                                           ctypes.c_size_t]
    lib.axon_start_nrt_profile.restype = ctypes.c_int64
    lib.axon_stop_nrt_profile.argtypes = [ctypes.c_char_p]
    lib.axon_stop_nrt_profile.restype = ctypes.c_int64

    jax.devices()
    # warm-up execution: loads the NEFF and aligns core dispatch so the
    # profiled run isn't polluted by first-run start skew
    bass2jax.run_bass_via_pjrt(nc, in_maps, n_cores=NCORES)
    neff_dir = tempfile.mkdtemp(prefix="bassprof_")
    rc = lib.axon_start_nrt_profile(None, 0)
    if rc != 0:
        raise RuntimeError(f"axon_start_nrt_profile rc={rc}")
    try:
        results = bass2jax.run_bass_via_pjrt(nc, in_maps, n_cores=NCORES)
    finally:
        n = lib.axon_stop_nrt_profile(neff_dir.encode())
        print(f"profile: {n} ntff file(s) in {neff_dir}")
    ntffs = glob.glob(neff_dir + "/*_body*.ntff")
    if not ntffs:
        return results, None, None
    profile = gauge.profiler.Profile(
        profile_path=FishPath(neff_dir), kernel_dev_mode=True,
        profile_on_exit=False, bass_kernel=nc.m,
        offline_processing=True, fname="*_body*")
    exec_ns = None
    try:
        prs = profile.to_perfetto(model_index=list(range(NCORES)))
        times = [pr.exec_time_ns for pr in prs if pr.exec_time_ns]
        exec_ns = max(times) if times else None
    except Exception as e:  # profile parse best-effort
        print("profile parse failed:", e)
    return results, exec_ns, neff_dir


def kernel(x, freqs_cis, wq, wk, wv, wo, q_norm_w, q_norm_b,
           k_norm_w, k_norm_b):
    global LAST_EXEC_NS, LAST_TRACE_DIR
    triv = (np.allclose(np.asarray(q_norm_w), 1.0)
            and np.allclose(np.asarray(q_norm_b), 0.0)
            and np.allclose(np.asarray(k_norm_w), 1.0)
            and np.allclose(np.asarray(k_norm_b), 0.0))
    nc = _build(triv)
    in_maps = _host_inputs(x, freqs_cis, wq, wk, wv, wo,
                           q_norm_w, q_norm_b, k_norm_w, k_norm_b, triv)
    if PROFILE:
        results, LAST_EXEC_NS, LAST_TRACE_DIR = _run_profiled(nc, in_maps)
    else:
        res = bass_utils.run_bass_kernel_spmd(
            nc, in_maps, core_ids=list(range(NCORES)))
        results = res.results
        LAST_EXEC_NS = res.exec_time_ns
    acc = np.zeros((T, D), np.float32)
    for r in results:
        acc += np.asarray(r["out"], np.float32)
    return acc.reshape(B, S, D)
